# revision 23
# baseline (speedup 1.0000x reference)
"""LinearAttention kernel for Trainium2, 8 NeuronCores, data-parallel over batch.

Reference (per batch, c=256 channels, n=4096 tokens):
  xn   = x / ||x||_c * g1 * 16
  qkv  = Wqkv @ xn            (q,k,v each [512, n])
  q    = softmax_d(q) / 8     (softmax over d=64 within each of 8 heads)
  k    = softmax_n(k)
  ctx_h = k_h @ v_h^T
  out  = Wout @ concat_h(ctx_h^T @ q_h) + bout
  out  = out / ||out||_c * g2 * 16

Sharding: 16 batches -> 8 cores x 2 batches. No collectives.

v4 design notes:
 - Projections in fp32r (1 cyc/row): fp8 in the value path costs ~4-6%
   relative error (random-sign contractions keep per-element quantization
   noise), blowing the 2e-2 budget.  fp8+DoubleRow only for the x^2
   channel-sum (positive sum, error averages).
 - Act uses only the natural_log_exp table set (preloaded explicitly
   once): rsqrt(s) = exp(-0.5*ln(s)) for both rms norms, exact exp for
   q and k, Square for the output norm. Zero table reloads.
 - Engines are in-order, so program order defines the pipeline: the two
   batches are tile-interleaved so both PSUM context banks accumulate
   concurrently and every engine sees work from the other batch while
   one batch waits on a dependency.
 - Streaming per-tile DMAs (in and out) instead of whole-batch buffers:
   loads never wait, stores trail each tile, and the SP sequencer is
   never head-of-line blocked on a cross-batch dependency.
 - Weight transpose/fold (g1 into Wqkv cols) on host; no device prologue.
 - Output stored bf16, upcast on host.
"""

import numpy as np

import concourse.bass as bass
import concourse.tile as tile
from concourse import bacc, mybir
from concourse.bass_utils import run_bass_kernel_spmd

F32 = mybir.dt.float32
F32R = mybir.dt.float32r
BF16 = mybir.dt.bfloat16
F8 = mybir.dt.float8e4
AF = mybir.ActivationFunctionType
OP = mybir.AluOpType
DR = mybir.MatmulPerfMode.DoubleRow

B = 16          # total batches
BL = 2          # batches per core
C = 256         # in channels
HID = 512       # heads * dim_head
HEADS = 8
DH = 64
N = 4096        # tokens
TN = 512        # token tile
NT = N // TN    # 8 tiles per batch
NB = TN // 128  # 4 128-token blocks per tile

ACT_TABLE_LN_EXP = 6  # index of natural_log_exp_and_others in act_func_sets


def build_kernel(with_bout: bool):
    nc = bacc.Bacc("TRN2", target_bir_lowering=False, debug=False, num_devices=8)

    x_d = nc.dram_tensor("x", [BL, C, N], BF16, kind="ExternalInput").ap()
    wq_d = nc.dram_tensor("wqkvT", [128, 2, 3 * HID], F32R, kind="ExternalInput").ap()
    wo_d = nc.dram_tensor("woutTb", [64, HEADS, C], F32R, kind="ExternalInput").ap()
    g2_d = nc.dram_tensor("g2c", [128, 2], F32, kind="ExternalInput").ap()
    if with_bout:
        bo_d = nc.dram_tensor("boutc", [128, 2], F32, kind="ExternalInput").ap()
    o_d = nc.dram_tensor("out", [BL, C, N], BF16, kind="ExternalOutput").ap()

    xv = x_d.rearrange("b (cb p) n -> b p cb n", cb=2)
    ov = o_d.rearrange("b (cb p) n -> b p cb n", cb=2)

    with tile.TileContext(nc) as tc:
        with (
            tc.tile_pool(name="const", bufs=1) as const,
            tc.tile_pool(name="big", bufs=1) as big,
            tc.tile_pool(name="work", bufs=1) as work,
            tc.tile_pool(name="ps", bufs=1, space="PSUM") as ps,
        ):
            # one activation-table load for the whole program
            nc.scalar.add_instruction(mybir.InstLoadActFuncSet(
                name=nc.get_next_instruction_name(), ins=[], outs=[],
                act_func_set_id=ACT_TABLE_LN_EXP))

            # ---- constants / weights (DMA straight into SBUF) ----
            wqkvT = const.tile([128, 2, 3 * HID], F32R)
            nc.sync.dma_start(out=wqkvT, in_=wq_d)
            woutTb = const.tile([64, HEADS, C], F32R)
            nc.sync.dma_start(out=woutTb, in_=wo_d)
            g2c = const.tile([128, 2], F32)
            nc.sync.dma_start(out=g2c, in_=g2_d)
            if with_bout:
                boutc = const.tile([128, 2], F32)
                nc.sync.dma_start(out=boutc, in_=bo_d)

            ones8 = const.tile([128, 2, 128], F8)
            nc.gpsimd.memset(ones8, 1.0)
            onesb = const.tile([128, 128], BF16)
            nc.gpsimd.memset(onesb, 1.0)
            bdb = const.tile([128, 128], BF16)
            nc.gpsimd.memset(bdb, 0.0)
            nc.gpsimd.memset(bdb[0:64, 0:64], 1.0)
            nc.gpsimd.memset(bdb[64:128, 64:128], 1.0)
            sclb = const.tile([1, 2], F32)
            nc.gpsimd.memset(sclb, 0.125)  # attention scale 1/8, via kdinv transpose

            # ---- per-batch persistent tensors ----
            q_sm = [None] * BL
            ctx_t = [None] * BL
            for bl in range(BL):
                q_sm[bl] = big.tile([128, 4, N], BF16, tag="qsm", bufs=2,
                                    name=f"qsm{bl}")
                ctx_t[bl] = ps.tile([128, 512], F32, tag="ctx", bufs=2,
                                    name=f"ctx{bl}")
                nc.vector.memset(ctx_t[bl], 0.0)

            # ====== batch epilogue: W2 = (Wout @ ctx^T / kden / 8)^T ======
            w2T = [None] * BL

            def epilogue(bl):
                kdinv = work.tile([1, 512], F32, tag="kdi", bufs=2)
                nc.vector.reciprocal(out=kdinv, in_=ctx_t[bl][64:65, :])
                ctx_sb = work.tile([64, 512], F32R, tag="ctxsb", bufs=2)
                nc.vector.tensor_copy(out=ctx_sb, in_=ctx_t[bl][0:64, :])
                pkd = ps.tile([128, 512], F32, tag="kv", bufs=1)
                for h in range(HEADS):
                    nc.tensor.matmul(
                        pkd[0:64, 2 * h:2 * h + 2],
                        kdinv[0:1, h * 64:(h + 1) * 64],
                        sclb, start=True, stop=True,
                    )
                kdcol = work.tile([64, HEADS, 1], F32, tag="kdcol", bufs=2)
                pkd_v = pkd[0:64, 0:16].rearrange("p (h t) -> p h t", t=2)
                nc.vector.tensor_copy(out=kdcol, in_=pkd_v[:, :, 0:1])
                w2T[bl] = work.tile([128, 4, 256], BF16, tag="w2T", bufs=2,
                                    name=f"w2T{bl}")
                for hh in range(2):
                    pw2 = ps.tile([64, 4, 256], F32, tag="pq", bufs=2)
                    for i in range(4):
                        h = hh * 4 + i
                        nc.tensor.matmul(
                            pw2[:, i, :],
                            ctx_sb[:, h * 64:(h + 1) * 64],
                            woutTb[:, h, :],
                            start=True, stop=True,
                        )
                    for i in range(4):
                        h = hh * 4 + i
                        nc.vector.tensor_scalar_mul(
                            out=w2T[bl][(h % 2) * 64:(h % 2) * 64 + 64, h // 2, :],
                            in0=pw2[:, i, :],
                            scalar1=kdcol[:, h, :],
                        )

            # ========== stage A: norm-prefix software-pipelined 1 tile ahead ==
            def prefix(j, bl):
                t0 = j * TN
                xin = work.tile([128, 2, TN], BF16, tag="xin", bufs=5)
                nc.sync.dma_start(out=xin, in_=xv[bl, :, :, t0:t0 + TN])
                # channel sum-of-squares -> 16/||x|| via exp(-0.5*ln(.))
                x2 = work.tile([128, 2, TN], F8, tag="x2", bufs=3)
                nc.gpsimd.tensor_mul(x2, xin, xin)
                ssq = ps.tile([128, 512], F32, tag="pq", bufs=2)
                nc.tensor.matmul(ssq, ones8, x2, start=True, stop=True,
                                 perf_mode=DR)
                lns = work.tile([128, TN], BF16, tag="lns", bufs=3)
                nc.scalar.activation(out=lns, in_=ssq, func=AF.Ln,
                                     scale=1.0 / 256.0)
                sinv = work.tile([128, TN], F32, tag="sinv", bufs=3)
                nc.scalar.activation(out=sinv, in_=lns, func=AF.Exp,
                                     scale=-0.5)
                xn = work.tile([128, 2, TN], F32R, tag="xn", bufs=5)
                for cb in range(2):
                    nc.gpsimd.tensor_mul(xn[:, cb, :], xin[:, cb, :], sinv)
                return xn

            def body_a(j, bl, xn):
                t0 = j * TN
                # q = Wq @ xn, fp32r
                eq = work.tile([128, 4, TN], BF16, tag="eq", bufs=3)
                qdi = work.tile([128, 4, TN], BF16, tag="qdi", bufs=3)
                pqs = []
                for half in range(2):
                    pq = ps.tile([128, 2, 512], F32, tag="pq", bufs=2)
                    pqs.append(pq)
                    for i in range(2):
                        ob = half * 2 + i
                        for cb in range(2):
                            nc.tensor.matmul(
                                pq[:, i, :],
                                wqkvT[:, cb, ob * 128:(ob + 1) * 128],
                                xn[:, cb, :],
                                start=(cb == 0), stop=(cb == 1),
                            )
                    nc.scalar.activation(out=eq[:, 2 * half:2 * half + 2, :],
                                         in_=pq, func=AF.Exp)

                def kv_block(nb):
                    pkv = ps.tile([128, 1024], F32, tag="kv", bufs=1)
                    for half in range(2):
                        for cb in range(2):
                            nc.tensor.matmul(
                                pkv[:, half * 512:(half + 1) * 512],
                                xn[:, cb, nb * 128:(nb + 1) * 128],
                                wqkvT[:, cb, HID + half * 512:
                                      HID + (half + 1) * 512],
                                start=(cb == 0), stop=(cb == 1),
                            )
                    ek = work.tile([128, HEADS, DH], BF16, tag="ek", bufs=6)
                    nc.scalar.activation(
                        out=ek.rearrange("p h d -> p (h d)"),
                        in_=pkv[:, 0:512], func=AF.Exp)
                    vt = work.tile([128, HEADS, 65], BF16, tag="vt", bufs=6)
                    nc.gpsimd.memset(vt[:, :, 64:65], 1.0)
                    vsrc = pkv[:, 512:1024].rearrange("p (h e) -> p h e", h=8)
                    nc.vector.tensor_copy(out=vt[:, :, 0:64], in_=vsrc)
                    return ek, vt

                def ctx_block(nb, ekvt):
                    ek, vt = ekvt
                    gnb = j * NB + nb
                    for h in range(HEADS):
                        nc.tensor.matmul(
                            ctx_t[bl][0:65, h * DH:(h + 1) * DH],
                            vt[:, h, :],
                            ek[:, h, :],
                            start=False, stop=(gnb == N // 128 - 1),
                            skip_group_check=True,
                        )

                kv01 = [kv_block(0), kv_block(1)]
                # q softmax denominator in-place over the q logits (post-exp)
                for half in range(2):
                    pd = pqs[half]
                    for i in range(2):
                        nc.tensor.matmul(pd[:, i, :], bdb,
                                         eq[:, 2 * half + i, :],
                                         start=True, stop=True)
                    with nc.allow_low_precision(reason="softmax recip bf16"):
                        nc.vector.reciprocal(
                            out=qdi[:, 2 * half:2 * half + 2, :], in_=pd)
                ctx_block(0, kv01[0])
                kv23 = [kv_block(2), kv_block(3)]
                nc.gpsimd.tensor_mul(q_sm[bl][:, :, t0:t0 + TN], eq, qdi)
                ctx_block(1, kv01[1])
                ctx_block(2, kv23[0])
                ctx_block(3, kv23[1])

            xns = {}
            for bl in range(BL):
                xns[(0, bl)] = prefix(0, bl)
            for j in range(NT):
                for bl in range(BL):
                    if j + 1 < NT:
                        xns[(j + 1, bl)] = prefix(j + 1, bl)
                    body_a(j, bl, xns.pop((j, bl)))
                    if j == NT - 1:
                        epilogue(bl)

            # ================= stage B (tile-interleaved batches) ==========
            po_n = [0]

            def po_block(j, bl):
                t0 = j * TN
                tag = "pq" if po_n[0] % 3 < 2 else "kv"
                po_n[0] += 1
                po = ps.tile([128, 2, 512], F32, tag=tag,
                             bufs=2 if tag == "pq" else 1)
                for ob in range(2):
                    for kb in range(4):
                        nc.tensor.matmul(
                            po[:, ob, :],
                            w2T[bl][:, kb, ob * 128:(ob + 1) * 128],
                            q_sm[bl][:, kb, t0:t0 + TN],
                            start=(kb == 0), stop=(kb == 3),
                        )
                return po

            def tail_b(j, bl, po):
                t0 = j * TN
                if with_bout:
                    yb = work.tile([128, 2, TN], F32, tag="yb", bufs=2)
                    for cb in range(2):
                        nc.vector.tensor_scalar_add(
                            out=yb[:, cb, :], in0=po[:, cb, :],
                            scalar1=boutc[:, cb:cb + 1])
                    ysrc = yb
                else:
                    ysrc = po
                y2 = work.tile([128, 2, TN], BF16, tag="y2", bufs=3)
                nc.scalar.activation(out=y2, in_=ysrc, func=AF.Square)
                ssqo = ps.tile([128, 512], F32, tag="ctx", bufs=2)
                for cb in range(2):
                    nc.tensor.matmul(ssqo, onesb, y2[:, cb, :],
                                     start=(cb == 0), stop=(cb == 1))
                lno = work.tile([128, TN], F32, tag="lno", bufs=3)
                nc.scalar.activation(out=lno, in_=ssqo, func=AF.Ln,
                                     scale=1.0 / 256.0)
                rgo = work.tile([128, TN], F32, tag="rgo", bufs=3)
                nc.scalar.activation(out=rgo, in_=lno, func=AF.Exp,
                                     scale=-0.5)
                outt = work.tile([128, 2, TN], BF16, tag="outt", bufs=6)
                for cb in range(2):
                    nc.vector.scalar_tensor_tensor(
                        out=outt[:, cb, :],
                        in0=ysrc[:, cb, :],
                        scalar=g2c[:, cb:cb + 1],
                        in1=rgo,
                        op0=OP.mult, op1=OP.mult,
                    )
                nc.sync.dma_start(out=ov[bl, :, :, t0:t0 + TN], in_=outt)

            for j in range(NT):
                pos = [po_block(j, bl) for bl in range(BL)]
                for bl in range(BL):
                    tail_b(j, bl, pos[bl])

    nc.finalize()
    return nc


_NC_CACHE = {}


def kernel(x, g1, Wqkv, Wout, bout, g2):
    x = np.ascontiguousarray(np.asarray(x, dtype=np.float32))
    g1 = np.asarray(g1, dtype=np.float32)
    Wqkv = np.asarray(Wqkv, dtype=np.float32)
    Wout = np.asarray(Wout, dtype=np.float32)
    bout = np.asarray(bout, dtype=np.float32)
    g2 = np.asarray(g2, dtype=np.float32)

    b, c, H, W = x.shape
    xr = x.reshape(b, c, H * W)

    bf = mybir.dt.np(BF16)
    # WqkvT [c, 3H] with g1 folded per channel, [p, cb, 3H] layout
    wqkvT = np.ascontiguousarray(
        (Wqkv.T * g1[:, None]).reshape(2, 128, 3 * HID).transpose(1, 0, 2)
    ).astype(np.float32)
    woutTb = np.ascontiguousarray(
        Wout.reshape(C, HEADS, DH).transpose(2, 1, 0)).astype(np.float32)
    g2c = np.ascontiguousarray(g2.reshape(2, 128).T)
    with_bout = bool(np.any(bout))

    if with_bout not in _NC_CACHE:
        _NC_CACHE[with_bout] = build_kernel(with_bout)
    nc = _NC_CACHE[with_bout]

    in_maps = []
    for core in range(8):
        m = {
            "x": np.ascontiguousarray(xr[core * BL:(core + 1) * BL].astype(bf)),
            "wqkvT": wqkvT, "woutTb": woutTb, "g2c": g2c,
        }
        if with_bout:
            m["boutc"] = np.ascontiguousarray(bout.reshape(2, 128).T)
        in_maps.append(m)
    res = run_bass_kernel_spmd(nc, in_maps, core_ids=list(range(8)))
    out = np.concatenate(
        [np.asarray(m["out"]).astype(np.float32) for m in res.results], axis=0)
    return out.reshape(b, c, H, W)


if __name__ == "__main__":
    rng = np.random.default_rng(0)
    inputs = dict(
        x=rng.standard_normal((16, 256, 64, 64), dtype=np.float32),
        g1=np.ones(256, np.float32),
        Wqkv=(rng.standard_normal((1536, 256), dtype=np.float32) * 256 ** -0.5),
        Wout=(rng.standard_normal((256, 512), dtype=np.float32) * 512 ** -0.5),
        bout=np.zeros(256, np.float32),
        g2=np.ones(256, np.float32),
    )
    out = kernel(**inputs)
    print("out", out.shape, out.dtype, np.abs(out).max())


# revision 24
# speedup vs baseline: 1.0275x; 1.0275x over previous
"""LinearAttention kernel for Trainium2, 8 NeuronCores, data-parallel over batch.

Reference (per batch, c=256 channels, n=4096 tokens):
  xn   = x / ||x||_c * g1 * 16
  qkv  = Wqkv @ xn            (q,k,v each [512, n])
  q    = softmax_d(q) / 8     (softmax over d=64 within each of 8 heads)
  k    = softmax_n(k)
  ctx_h = k_h @ v_h^T
  out  = Wout @ concat_h(ctx_h^T @ q_h) + bout
  out  = out / ||out||_c * g2 * 16

Sharding: 16 batches -> 8 cores x 2 batches. No collectives.

v4 design notes:
 - Projections in fp32r (1 cyc/row): fp8 in the value path costs ~4-6%
   relative error (random-sign contractions keep per-element quantization
   noise), blowing the 2e-2 budget.  fp8+DoubleRow only for the x^2
   channel-sum (positive sum, error averages).
 - Act uses only the natural_log_exp table set (preloaded explicitly
   once): rsqrt(s) = exp(-0.5*ln(s)) for both rms norms, exact exp for
   q and k, Square for the output norm. Zero table reloads.
 - Engines are in-order, so program order defines the pipeline: the two
   batches are tile-interleaved so both PSUM context banks accumulate
   concurrently and every engine sees work from the other batch while
   one batch waits on a dependency.
 - Streaming per-tile DMAs (in and out) instead of whole-batch buffers:
   loads never wait, stores trail each tile, and the SP sequencer is
   never head-of-line blocked on a cross-batch dependency.
 - Weight transpose/fold (g1 into Wqkv cols) on host; no device prologue.
 - Output stored bf16, upcast on host.
"""

import numpy as np

import concourse.bass as bass
import concourse.tile as tile
from concourse import bacc, mybir
from concourse.bass_utils import run_bass_kernel_spmd

F32 = mybir.dt.float32
F32R = mybir.dt.float32r
BF16 = mybir.dt.bfloat16
F8 = mybir.dt.float8e4
AF = mybir.ActivationFunctionType
OP = mybir.AluOpType
DR = mybir.MatmulPerfMode.DoubleRow

B = 16          # total batches
BL = 2          # batches per core
C = 256         # in channels
HID = 512       # heads * dim_head
HEADS = 8
DH = 64
N = 4096        # tokens
TN = 512        # token tile
NT = N // TN    # 8 tiles per batch
NB = TN // 128  # 4 128-token blocks per tile

ACT_TABLE_LN_EXP = 6  # index of natural_log_exp_and_others in act_func_sets


def build_kernel(with_bout: bool):
    nc = bacc.Bacc("TRN2", target_bir_lowering=False, debug=False, num_devices=8)

    x_d = nc.dram_tensor("x", [BL, C, N], BF16, kind="ExternalInput").ap()
    wq_d = nc.dram_tensor("wqkvT", [128, 2, 3 * HID], F32R, kind="ExternalInput").ap()
    wo_d = nc.dram_tensor("woutTb", [64, HEADS, C], F32R, kind="ExternalInput").ap()
    g2_d = nc.dram_tensor("g2c", [128, 2], F32, kind="ExternalInput").ap()
    if with_bout:
        bo_d = nc.dram_tensor("boutc", [128, 2], F32, kind="ExternalInput").ap()
    o_d = nc.dram_tensor("out", [BL, C, N], BF16, kind="ExternalOutput").ap()

    xv = x_d.rearrange("b (cb p) n -> b p cb n", cb=2)
    ov = o_d.rearrange("b (cb p) n -> b p cb n", cb=2)

    with tile.TileContext(nc) as tc:
        with (
            tc.tile_pool(name="const", bufs=1) as const,
            tc.tile_pool(name="big", bufs=1) as big,
            tc.tile_pool(name="work", bufs=1) as work,
            tc.tile_pool(name="ps", bufs=1, space="PSUM") as ps,
        ):
            # one activation-table load for the whole program
            nc.scalar.add_instruction(mybir.InstLoadActFuncSet(
                name=nc.get_next_instruction_name(), ins=[], outs=[],
                act_func_set_id=ACT_TABLE_LN_EXP))

            # ---- constants / weights (DMA straight into SBUF) ----
            wqkvT = const.tile([128, 2, 3 * HID], F32R)
            nc.sync.dma_start(out=wqkvT, in_=wq_d)
            woutTb = const.tile([64, HEADS, C], F32R)
            nc.sync.dma_start(out=woutTb, in_=wo_d)
            g2c = const.tile([128, 2], F32)
            nc.sync.dma_start(out=g2c, in_=g2_d)
            if with_bout:
                boutc = const.tile([128, 2], F32)
                nc.sync.dma_start(out=boutc, in_=bo_d)

            ones8 = const.tile([128, 2, 128], F8)
            nc.gpsimd.memset(ones8, 1.0)
            onesb = const.tile([128, 128], BF16)
            nc.gpsimd.memset(onesb, 1.0)
            bdb = const.tile([128, 128], BF16)
            nc.gpsimd.memset(bdb, 0.0)
            nc.gpsimd.memset(bdb[0:64, 0:64], 1.0)
            nc.gpsimd.memset(bdb[64:128, 64:128], 1.0)
            sclb = const.tile([1, 2], F32)
            nc.gpsimd.memset(sclb, 0.125)  # attention scale 1/8, via kdinv transpose

            # ---- per-batch persistent tensors ----
            q_sm = [None] * BL
            ctx_t = [None] * BL
            for bl in range(BL):
                q_sm[bl] = big.tile([128, 4, N], BF16, tag="qsm", bufs=2,
                                    name=f"qsm{bl}")
                ctx_t[bl] = ps.tile([128, 512], F32, tag="ctx", bufs=2,
                                    name=f"ctx{bl}")
                nc.vector.memset(ctx_t[bl], 0.0)

            # ====== batch epilogue: W2 = (Wout @ ctx^T / kden / 8)^T ======
            w2T = [None] * BL

            def epilogue(bl):
                kdinv = work.tile([1, 512], F32, tag="kdi", bufs=2)
                nc.vector.reciprocal(out=kdinv, in_=ctx_t[bl][64:65, :])
                ctx_sb = work.tile([64, 512], F32R, tag="ctxsb", bufs=2)
                nc.vector.tensor_copy(out=ctx_sb, in_=ctx_t[bl][0:64, :])
                pkd = ps.tile([128, 512], F32, tag="kv", bufs=1)
                for h in range(HEADS):
                    nc.tensor.matmul(
                        pkd[0:64, 2 * h:2 * h + 2],
                        kdinv[0:1, h * 64:(h + 1) * 64],
                        sclb, start=True, stop=True,
                    )
                kdcol = work.tile([64, HEADS, 1], F32, tag="kdcol", bufs=2)
                pkd_v = pkd[0:64, 0:16].rearrange("p (h t) -> p h t", t=2)
                nc.vector.tensor_copy(out=kdcol, in_=pkd_v[:, :, 0:1])
                w2T[bl] = work.tile([128, 4, 256], BF16, tag="w2T", bufs=2,
                                    name=f"w2T{bl}")
                for hh in range(2):
                    pw2 = ps.tile([64, 4, 256], F32, tag="pq", bufs=2)
                    for i in range(4):
                        h = hh * 4 + i
                        nc.tensor.matmul(
                            pw2[:, i, :],
                            ctx_sb[:, h * 64:(h + 1) * 64],
                            woutTb[:, h, :],
                            start=True, stop=True,
                        )
                    for i in range(4):
                        h = hh * 4 + i
                        nc.vector.tensor_scalar_mul(
                            out=w2T[bl][(h % 2) * 64:(h % 2) * 64 + 64, h // 2, :],
                            in0=pw2[:, i, :],
                            scalar1=kdcol[:, h, :],
                        )

            # ========== stage A: norm-prefix software-pipelined 1 tile ahead ==
            def prefix(j, bl):
                t0 = j * TN
                xin = work.tile([128, 2, TN], BF16, tag="xin", bufs=5)
                nc.sync.dma_start(out=xin, in_=xv[bl, :, :, t0:t0 + TN])
                # channel sum-of-squares -> 16/||x|| via exp(-0.5*ln(.))
                x2 = work.tile([128, 2, TN], F8, tag="x2", bufs=3)
                nc.gpsimd.tensor_mul(x2, xin, xin)
                ssq = ps.tile([128, 512], F32, tag="pq", bufs=2)
                nc.tensor.matmul(ssq, ones8, x2, start=True, stop=True,
                                 perf_mode=DR)
                lns = work.tile([128, TN], BF16, tag="lns", bufs=3)
                nc.scalar.activation(out=lns, in_=ssq, func=AF.Ln,
                                     scale=1.0 / 256.0)
                sinv = work.tile([128, TN], F32, tag="sinv", bufs=3)
                nc.scalar.activation(out=sinv, in_=lns, func=AF.Exp,
                                     scale=-0.5)
                xn = work.tile([128, 2, TN], F32R, tag="xn", bufs=5)
                for cb in range(2):
                    nc.gpsimd.tensor_mul(xn[:, cb, :], xin[:, cb, :], sinv)
                return xn

            def body_a(j, bl, xn):
                t0 = j * TN
                # q = Wq @ xn, fp32r
                eq = work.tile([128, 4, TN], BF16, tag="eq", bufs=3)
                qdi = work.tile([128, 4, TN], BF16, tag="qdi", bufs=3)
                pqs = []
                for half in range(2):
                    pq = ps.tile([128, 2, 512], F32, tag="pq", bufs=2)
                    pqs.append(pq)
                    for i in range(2):
                        ob = half * 2 + i
                        for cb in range(2):
                            nc.tensor.matmul(
                                pq[:, i, :],
                                wqkvT[:, cb, ob * 128:(ob + 1) * 128],
                                xn[:, cb, :],
                                start=(cb == 0), stop=(cb == 1),
                            )
                    nc.scalar.activation(out=eq[:, 2 * half:2 * half + 2, :],
                                         in_=pq, func=AF.Exp)

                def kv_block(nb):
                    pkv = ps.tile([128, 1024], F32, tag="kv", bufs=1)
                    for half in range(2):
                        for cb in range(2):
                            nc.tensor.matmul(
                                pkv[:, half * 512:(half + 1) * 512],
                                xn[:, cb, nb * 128:(nb + 1) * 128],
                                wqkvT[:, cb, HID + half * 512:
                                      HID + (half + 1) * 512],
                                start=(cb == 0), stop=(cb == 1),
                            )
                    ek = work.tile([128, HEADS, DH], BF16, tag="ek", bufs=6)
                    nc.scalar.activation(
                        out=ek.rearrange("p h d -> p (h d)"),
                        in_=pkv[:, 0:512], func=AF.Exp)
                    vt = work.tile([128, HEADS, 65], BF16, tag="vt", bufs=6)
                    nc.gpsimd.memset(vt[:, :, 64:65], 1.0)
                    vsrc = pkv[:, 512:1024].rearrange("p (h e) -> p h e", h=8)
                    nc.vector.tensor_copy(out=vt[:, :, 0:64], in_=vsrc)
                    return ek, vt

                def ctx_block(nb, ekvt):
                    ek, vt = ekvt
                    gnb = j * NB + nb
                    for h in range(HEADS):
                        nc.tensor.matmul(
                            ctx_t[bl][0:65, h * DH:(h + 1) * DH],
                            vt[:, h, :],
                            ek[:, h, :],
                            start=False, stop=(gnb == N // 128 - 1),
                            skip_group_check=True,
                        )

                kv01 = [kv_block(0), kv_block(1)]
                # q softmax denominator in-place over the q logits (post-exp)
                for half in range(2):
                    pd = pqs[half]
                    for i in range(2):
                        nc.tensor.matmul(pd[:, i, :], bdb,
                                         eq[:, 2 * half + i, :],
                                         start=True, stop=True)
                    with nc.allow_low_precision(reason="softmax recip bf16"):
                        nc.vector.reciprocal(
                            out=qdi[:, 2 * half:2 * half + 2, :], in_=pd)
                ctx_block(0, kv01[0])
                kv23 = [kv_block(2), kv_block(3)]
                nc.vector.tensor_mul(q_sm[bl][:, :, t0:t0 + TN], eq, qdi)
                ctx_block(1, kv01[1])
                ctx_block(2, kv23[0])
                ctx_block(3, kv23[1])

            xns = {}
            for bl in range(BL):
                xns[(0, bl)] = prefix(0, bl)
            for j in range(NT):
                for bl in range(BL):
                    if j + 1 < NT:
                        xns[(j + 1, bl)] = prefix(j + 1, bl)
                    body_a(j, bl, xns.pop((j, bl)))
                    if j == NT - 1:
                        epilogue(bl)

            # ================= stage B (tile-interleaved batches) ==========
            po_n = [0]

            def po_block(j, bl):
                t0 = j * TN
                tag = "pq" if po_n[0] % 3 < 2 else "kv"
                po_n[0] += 1
                po = ps.tile([128, 2, 512], F32, tag=tag,
                             bufs=2 if tag == "pq" else 1)
                for ob in range(2):
                    for kb in range(4):
                        nc.tensor.matmul(
                            po[:, ob, :],
                            w2T[bl][:, kb, ob * 128:(ob + 1) * 128],
                            q_sm[bl][:, kb, t0:t0 + TN],
                            start=(kb == 0), stop=(kb == 3),
                        )
                return po

            def tail_b(j, bl, po):
                t0 = j * TN
                if with_bout:
                    yb = work.tile([128, 2, TN], F32, tag="yb", bufs=2)
                    for cb in range(2):
                        nc.vector.tensor_scalar_add(
                            out=yb[:, cb, :], in0=po[:, cb, :],
                            scalar1=boutc[:, cb:cb + 1])
                    ysrc = yb
                else:
                    ysrc = po
                y2 = work.tile([128, 2, TN], BF16, tag="y2", bufs=3)
                nc.scalar.activation(out=y2, in_=ysrc, func=AF.Square)
                ssqo = ps.tile([128, 512], F32, tag="ctx", bufs=2)
                for cb in range(2):
                    nc.tensor.matmul(ssqo, onesb, y2[:, cb, :],
                                     start=(cb == 0), stop=(cb == 1))
                lno = work.tile([128, TN], F32, tag="lno", bufs=3)
                nc.scalar.activation(out=lno, in_=ssqo, func=AF.Ln,
                                     scale=1.0 / 256.0)
                rgo = work.tile([128, TN], F32, tag="rgo", bufs=3)
                nc.scalar.activation(out=rgo, in_=lno, func=AF.Exp,
                                     scale=-0.5)
                outt = work.tile([128, 2, TN], BF16, tag="outt", bufs=6)
                for cb in range(2):
                    nc.vector.scalar_tensor_tensor(
                        out=outt[:, cb, :],
                        in0=ysrc[:, cb, :],
                        scalar=g2c[:, cb:cb + 1],
                        in1=rgo,
                        op0=OP.mult, op1=OP.mult,
                    )
                nc.sync.dma_start(out=ov[bl, :, :, t0:t0 + TN], in_=outt)

            for j in range(NT):
                pos = [po_block(j, bl) for bl in range(BL)]
                for bl in range(BL):
                    tail_b(j, bl, pos[bl])

    nc.finalize()
    return nc


_NC_CACHE = {}


def kernel(x, g1, Wqkv, Wout, bout, g2):
    x = np.ascontiguousarray(np.asarray(x, dtype=np.float32))
    g1 = np.asarray(g1, dtype=np.float32)
    Wqkv = np.asarray(Wqkv, dtype=np.float32)
    Wout = np.asarray(Wout, dtype=np.float32)
    bout = np.asarray(bout, dtype=np.float32)
    g2 = np.asarray(g2, dtype=np.float32)

    b, c, H, W = x.shape
    xr = x.reshape(b, c, H * W)

    bf = mybir.dt.np(BF16)
    # WqkvT [c, 3H] with g1 folded per channel, [p, cb, 3H] layout
    wqkvT = np.ascontiguousarray(
        (Wqkv.T * g1[:, None]).reshape(2, 128, 3 * HID).transpose(1, 0, 2)
    ).astype(np.float32)
    woutTb = np.ascontiguousarray(
        Wout.reshape(C, HEADS, DH).transpose(2, 1, 0)).astype(np.float32)
    g2c = np.ascontiguousarray(g2.reshape(2, 128).T)
    with_bout = bool(np.any(bout))

    if with_bout not in _NC_CACHE:
        _NC_CACHE[with_bout] = build_kernel(with_bout)
    nc = _NC_CACHE[with_bout]

    in_maps = []
    for core in range(8):
        m = {
            "x": np.ascontiguousarray(xr[core * BL:(core + 1) * BL].astype(bf)),
            "wqkvT": wqkvT, "woutTb": woutTb, "g2c": g2c,
        }
        if with_bout:
            m["boutc"] = np.ascontiguousarray(bout.reshape(2, 128).T)
        in_maps.append(m)
    res = run_bass_kernel_spmd(nc, in_maps, core_ids=list(range(8)))
    out = np.concatenate(
        [np.asarray(m["out"]).astype(np.float32) for m in res.results], axis=0)
    return out.reshape(b, c, H, W)


if __name__ == "__main__":
    rng = np.random.default_rng(0)
    inputs = dict(
        x=rng.standard_normal((16, 256, 64, 64), dtype=np.float32),
        g1=np.ones(256, np.float32),
        Wqkv=(rng.standard_normal((1536, 256), dtype=np.float32) * 256 ** -0.5),
        Wout=(rng.standard_normal((256, 512), dtype=np.float32) * 512 ** -0.5),
        bout=np.zeros(256, np.float32),
        g2=np.ones(256, np.float32),
    )
    out = kernel(**inputs)
    print("out", out.shape, out.dtype, np.abs(out).max())


# revision 25
# speedup vs baseline: 1.0331x; 1.0054x over previous
"""LinearAttention kernel for Trainium2, 8 NeuronCores, data-parallel over batch.

Reference (per batch, c=256 channels, n=4096 tokens):
  xn   = x / ||x||_c * g1 * 16
  qkv  = Wqkv @ xn            (q,k,v each [512, n])
  q    = softmax_d(q) / 8     (softmax over d=64 within each of 8 heads)
  k    = softmax_n(k)
  ctx_h = k_h @ v_h^T
  out  = Wout @ concat_h(ctx_h^T @ q_h) + bout
  out  = out / ||out||_c * g2 * 16

Sharding: 16 batches -> 8 cores x 2 batches. No collectives.

v4 design notes:
 - Projections in fp32r (1 cyc/row): fp8 in the value path costs ~4-6%
   relative error (random-sign contractions keep per-element quantization
   noise), blowing the 2e-2 budget.  fp8+DoubleRow only for the x^2
   channel-sum (positive sum, error averages).
 - Act uses only the natural_log_exp table set (preloaded explicitly
   once): rsqrt(s) = exp(-0.5*ln(s)) for both rms norms, exact exp for
   q and k, Square for the output norm. Zero table reloads.
 - Engines are in-order, so program order defines the pipeline: the two
   batches are tile-interleaved so both PSUM context banks accumulate
   concurrently and every engine sees work from the other batch while
   one batch waits on a dependency.
 - Streaming per-tile DMAs (in and out) instead of whole-batch buffers:
   loads never wait, stores trail each tile, and the SP sequencer is
   never head-of-line blocked on a cross-batch dependency.
 - Weight transpose/fold (g1 into Wqkv cols) on host; no device prologue.
 - Output stored bf16, upcast on host.
"""

import numpy as np

import concourse.bass as bass
import concourse.tile as tile
from concourse import bacc, mybir
from concourse.bass_utils import run_bass_kernel_spmd

F32 = mybir.dt.float32
F32R = mybir.dt.float32r
BF16 = mybir.dt.bfloat16
F8 = mybir.dt.float8e4
AF = mybir.ActivationFunctionType
OP = mybir.AluOpType
DR = mybir.MatmulPerfMode.DoubleRow

B = 16          # total batches
BL = 2          # batches per core
C = 256         # in channels
HID = 512       # heads * dim_head
HEADS = 8
DH = 64
N = 4096        # tokens
TN = 512        # token tile
NT = N // TN    # 8 tiles per batch
NB = TN // 128  # 4 128-token blocks per tile

ACT_TABLE_LN_EXP = 6  # index of natural_log_exp_and_others in act_func_sets


def build_kernel(with_bout: bool):
    nc = bacc.Bacc("TRN2", target_bir_lowering=False, debug=False, num_devices=8)

    x_d = nc.dram_tensor("x", [BL, C, N], BF16, kind="ExternalInput").ap()
    wq_d = nc.dram_tensor("wqkvT", [128, 2, 3 * HID], F32R, kind="ExternalInput").ap()
    wo_d = nc.dram_tensor("woutTb", [64, HEADS, C], F32R, kind="ExternalInput").ap()
    g2_d = nc.dram_tensor("g2c", [128, 2], F32, kind="ExternalInput").ap()
    if with_bout:
        bo_d = nc.dram_tensor("boutc", [128, 2], F32, kind="ExternalInput").ap()
    o_d = nc.dram_tensor("out", [BL, C, N], BF16, kind="ExternalOutput").ap()

    xv = x_d.rearrange("b (cb p) n -> b p cb n", cb=2)
    ov = o_d.rearrange("b (cb p) n -> b p cb n", cb=2)

    with tile.TileContext(nc) as tc:
        with (
            tc.tile_pool(name="const", bufs=1) as const,
            tc.tile_pool(name="big", bufs=1) as big,
            tc.tile_pool(name="work", bufs=1) as work,
            tc.tile_pool(name="ps", bufs=1, space="PSUM") as ps,
        ):
            # one activation-table load for the whole program
            nc.scalar.add_instruction(mybir.InstLoadActFuncSet(
                name=nc.get_next_instruction_name(), ins=[], outs=[],
                act_func_set_id=ACT_TABLE_LN_EXP))

            # ---- constants / weights (DMA straight into SBUF) ----
            wqkvT = const.tile([128, 2, 3 * HID], F32R)
            nc.sync.dma_start(out=wqkvT, in_=wq_d)
            woutTb = const.tile([64, HEADS, C], F32R)
            nc.sync.dma_start(out=woutTb, in_=wo_d)
            g2c = const.tile([128, 2], F32)
            nc.sync.dma_start(out=g2c, in_=g2_d)
            if with_bout:
                boutc = const.tile([128, 2], F32)
                nc.sync.dma_start(out=boutc, in_=bo_d)

            ones8 = const.tile([128, 2, 128], F8)
            nc.gpsimd.memset(ones8, 1.0)
            onesb = const.tile([128, 128], BF16)
            nc.gpsimd.memset(onesb, 1.0)
            bdb = const.tile([128, 128], BF16)
            nc.gpsimd.memset(bdb, 0.0)
            nc.gpsimd.memset(bdb[0:64, 0:64], 1.0)
            nc.gpsimd.memset(bdb[64:128, 64:128], 1.0)
            sclb = const.tile([1, 2], F32)
            nc.gpsimd.memset(sclb, 0.125)  # attention scale 1/8, via kdinv transpose

            # ---- per-batch persistent tensors ----
            q_sm = [None] * BL
            ctx_t = [None] * BL
            for bl in range(BL):
                q_sm[bl] = big.tile([128, 4, N], BF16, tag="qsm", bufs=2,
                                    name=f"qsm{bl}")
                ctx_t[bl] = ps.tile([128, 512], F32, tag="ctx", bufs=2,
                                    name=f"ctx{bl}")
                nc.vector.memset(ctx_t[bl], 0.0)

            # ====== batch epilogue: W2 = (Wout @ ctx^T / kden / 8)^T ======
            w2T = [None] * BL

            def epilogue(bl):
                kdinv = work.tile([1, 512], F32, tag="kdi", bufs=2)
                nc.vector.reciprocal(out=kdinv, in_=ctx_t[bl][64:65, :])
                ctx_sb = work.tile([64, 512], F32R, tag="ctxsb", bufs=2)
                nc.vector.tensor_copy(out=ctx_sb, in_=ctx_t[bl][0:64, :])
                pkd = ps.tile([128, 512], F32, tag="kv", bufs=1)
                for h in range(HEADS):
                    nc.tensor.matmul(
                        pkd[0:64, 2 * h:2 * h + 2],
                        kdinv[0:1, h * 64:(h + 1) * 64],
                        sclb, start=True, stop=True,
                    )
                kdcol = work.tile([64, HEADS, 1], F32, tag="kdcol", bufs=2)
                pkd_v = pkd[0:64, 0:16].rearrange("p (h t) -> p h t", t=2)
                nc.vector.tensor_copy(out=kdcol, in_=pkd_v[:, :, 0:1])
                w2T[bl] = work.tile([128, 4, 256], BF16, tag="w2T", bufs=2,
                                    name=f"w2T{bl}")
                for hh in range(2):
                    pw2 = ps.tile([64, 4, 256], F32, tag="pq", bufs=2)
                    for i in range(4):
                        h = hh * 4 + i
                        nc.tensor.matmul(
                            pw2[:, i, :],
                            ctx_sb[:, h * 64:(h + 1) * 64],
                            woutTb[:, h, :],
                            start=True, stop=True,
                        )
                    for i in range(4):
                        h = hh * 4 + i
                        nc.vector.tensor_scalar_mul(
                            out=w2T[bl][(h % 2) * 64:(h % 2) * 64 + 64, h // 2, :],
                            in0=pw2[:, i, :],
                            scalar1=kdcol[:, h, :],
                        )

            # ========== stage A: norm-prefix software-pipelined 1 tile ahead ==
            def prefix(j, bl):
                t0 = j * TN
                xin = work.tile([128, 2, TN], BF16, tag="xin", bufs=5)
                nc.sync.dma_start(out=xin, in_=xv[bl, :, :, t0:t0 + TN])
                # channel sum-of-squares -> 16/||x|| via exp(-0.5*ln(.))
                x2 = work.tile([128, 2, TN], F8, tag="x2", bufs=3)
                nc.gpsimd.tensor_mul(x2, xin, xin)
                ssq = ps.tile([128, 512], F32, tag="pq", bufs=2)
                nc.tensor.matmul(ssq, ones8, x2, start=True, stop=True,
                                 perf_mode=DR)
                lns = work.tile([128, TN], BF16, tag="lns", bufs=3)
                nc.scalar.activation(out=lns, in_=ssq, func=AF.Ln,
                                     scale=1.0 / 256.0)
                sinv = work.tile([128, TN], F32, tag="sinv", bufs=3)
                nc.scalar.activation(out=sinv, in_=lns, func=AF.Exp,
                                     scale=-0.5)
                xn = work.tile([128, 2, TN], F32R, tag="xn", bufs=5)
                for cb in range(2):
                    nc.gpsimd.tensor_mul(xn[:, cb, :], xin[:, cb, :], sinv)
                return xn

            def body_a(j, bl, xn):
                t0 = j * TN
                # q = Wq @ xn, fp32r
                eq = work.tile([128, 4, TN], BF16, tag="eq", bufs=3)
                qdi = work.tile([128, 4, TN], BF16, tag="qdi", bufs=3)
                pqs = []
                for half in range(2):
                    pq = ps.tile([128, 2, 512], F32, tag="pq", bufs=2)
                    pqs.append(pq)
                    for i in range(2):
                        ob = half * 2 + i
                        for cb in range(2):
                            nc.tensor.matmul(
                                pq[:, i, :],
                                wqkvT[:, cb, ob * 128:(ob + 1) * 128],
                                xn[:, cb, :],
                                start=(cb == 0), stop=(cb == 1),
                            )
                    nc.scalar.activation(out=eq[:, 2 * half:2 * half + 2, :],
                                         in_=pq, func=AF.Exp)

                def kv_block(nb):
                    pkv = ps.tile([128, 1024], F32, tag="kv", bufs=1)
                    for half in range(2):
                        for cb in range(2):
                            nc.tensor.matmul(
                                pkv[:, half * 512:(half + 1) * 512],
                                xn[:, cb, nb * 128:(nb + 1) * 128],
                                wqkvT[:, cb, HID + half * 512:
                                      HID + (half + 1) * 512],
                                start=(cb == 0), stop=(cb == 1),
                            )
                    ek = work.tile([128, HEADS, DH], BF16, tag="ek", bufs=6)
                    nc.scalar.activation(
                        out=ek.rearrange("p h d -> p (h d)"),
                        in_=pkv[:, 0:512], func=AF.Exp)
                    vt = work.tile([128, HEADS, 65], BF16, tag="vt", bufs=6)
                    nc.gpsimd.memset(vt[:, :, 64:65], 1.0)
                    vsrc = pkv[:, 512:1024].rearrange("p (h e) -> p h e", h=8)
                    nc.vector.tensor_copy(out=vt[:, :, 0:64], in_=vsrc)
                    return ek, vt

                def ctx_block(nb, ekvt):
                    ek, vt = ekvt
                    gnb = j * NB + nb
                    for h in range(HEADS):
                        nc.tensor.matmul(
                            ctx_t[bl][0:65, h * DH:(h + 1) * DH],
                            vt[:, h, :],
                            ek[:, h, :],
                            start=False, stop=(gnb == N // 128 - 1),
                            skip_group_check=True,
                        )

                kv01 = [kv_block(0), kv_block(1)]
                # q softmax denominator while kv 2/3 project
                for half in range(2):
                    pd = ps.tile([128, 2, 512], F32, tag="pq", bufs=2)
                    for i in range(2):
                        nc.tensor.matmul(pd[:, i, :], bdb,
                                         eq[:, 2 * half + i, :],
                                         start=True, stop=True)
                    with nc.allow_low_precision(reason="softmax recip bf16"):
                        nc.vector.reciprocal(
                            out=qdi[:, 2 * half:2 * half + 2, :], in_=pd)
                ctx_block(0, kv01[0])
                kv23 = [kv_block(2), kv_block(3)]
                nc.vector.tensor_mul(q_sm[bl][:, :, t0:t0 + TN], eq, qdi)
                ctx_block(1, kv01[1])
                ctx_block(2, kv23[0])
                ctx_block(3, kv23[1])

            xns = {}
            for bl in range(BL):
                xns[(0, bl)] = prefix(0, bl)
            for j in range(NT):
                for bl in range(BL):
                    if j + 1 < NT:
                        xns[(j + 1, bl)] = prefix(j + 1, bl)
                    body_a(j, bl, xns.pop((j, bl)))
                    if j == NT - 1:
                        epilogue(bl)

            # ================= stage B (tile-interleaved batches) ==========
            po_n = [0]

            def po_block(j, bl):
                t0 = j * TN
                tag = "pq" if po_n[0] % 3 < 2 else "kv"
                po_n[0] += 1
                po = ps.tile([128, 2, 512], F32, tag=tag,
                             bufs=2 if tag == "pq" else 1)
                for ob in range(2):
                    for kb in range(4):
                        nc.tensor.matmul(
                            po[:, ob, :],
                            w2T[bl][:, kb, ob * 128:(ob + 1) * 128],
                            q_sm[bl][:, kb, t0:t0 + TN],
                            start=(kb == 0), stop=(kb == 3),
                        )
                return po

            def tail_b(j, bl, po):
                t0 = j * TN
                if with_bout:
                    yb = work.tile([128, 2, TN], F32, tag="yb", bufs=2)
                    for cb in range(2):
                        nc.vector.tensor_scalar_add(
                            out=yb[:, cb, :], in0=po[:, cb, :],
                            scalar1=boutc[:, cb:cb + 1])
                    ysrc = yb
                else:
                    ysrc = po
                y2 = work.tile([128, 2, TN], BF16, tag="y2", bufs=3)
                nc.scalar.activation(out=y2, in_=ysrc, func=AF.Square)
                ssqo = ps.tile([128, 512], F32, tag="ctx", bufs=2)
                for cb in range(2):
                    nc.tensor.matmul(ssqo, onesb, y2[:, cb, :],
                                     start=(cb == 0), stop=(cb == 1))
                lno = work.tile([128, TN], F32, tag="lno", bufs=3)
                nc.scalar.activation(out=lno, in_=ssqo, func=AF.Ln,
                                     scale=1.0 / 256.0)
                rgo = work.tile([128, TN], F32, tag="rgo", bufs=3)
                nc.scalar.activation(out=rgo, in_=lno, func=AF.Exp,
                                     scale=-0.5)
                outt = work.tile([128, 2, TN], BF16, tag="outt", bufs=6)
                for cb in range(2):
                    nc.vector.scalar_tensor_tensor(
                        out=outt[:, cb, :],
                        in0=ysrc[:, cb, :],
                        scalar=g2c[:, cb:cb + 1],
                        in1=rgo,
                        op0=OP.mult, op1=OP.mult,
                    )
                nc.sync.dma_start(out=ov[bl, :, :, t0:t0 + TN], in_=outt)

            for j in range(NT):
                pos = [po_block(j, bl) for bl in range(BL)]
                for bl in range(BL):
                    tail_b(j, bl, pos[bl])

    nc.finalize()
    return nc


_NC_CACHE = {}


def kernel(x, g1, Wqkv, Wout, bout, g2):
    x = np.ascontiguousarray(np.asarray(x, dtype=np.float32))
    g1 = np.asarray(g1, dtype=np.float32)
    Wqkv = np.asarray(Wqkv, dtype=np.float32)
    Wout = np.asarray(Wout, dtype=np.float32)
    bout = np.asarray(bout, dtype=np.float32)
    g2 = np.asarray(g2, dtype=np.float32)

    b, c, H, W = x.shape
    xr = x.reshape(b, c, H * W)

    bf = mybir.dt.np(BF16)
    # WqkvT [c, 3H] with g1 folded per channel, [p, cb, 3H] layout
    wqkvT = np.ascontiguousarray(
        (Wqkv.T * g1[:, None]).reshape(2, 128, 3 * HID).transpose(1, 0, 2)
    ).astype(np.float32)
    woutTb = np.ascontiguousarray(
        Wout.reshape(C, HEADS, DH).transpose(2, 1, 0)).astype(np.float32)
    g2c = np.ascontiguousarray(g2.reshape(2, 128).T)
    with_bout = bool(np.any(bout))

    if with_bout not in _NC_CACHE:
        _NC_CACHE[with_bout] = build_kernel(with_bout)
    nc = _NC_CACHE[with_bout]

    in_maps = []
    for core in range(8):
        m = {
            "x": np.ascontiguousarray(xr[core * BL:(core + 1) * BL].astype(bf)),
            "wqkvT": wqkvT, "woutTb": woutTb, "g2c": g2c,
        }
        if with_bout:
            m["boutc"] = np.ascontiguousarray(bout.reshape(2, 128).T)
        in_maps.append(m)
    res = run_bass_kernel_spmd(nc, in_maps, core_ids=list(range(8)))
    out = np.concatenate(
        [np.asarray(m["out"]).astype(np.float32) for m in res.results], axis=0)
    return out.reshape(b, c, H, W)


if __name__ == "__main__":
    rng = np.random.default_rng(0)
    inputs = dict(
        x=rng.standard_normal((16, 256, 64, 64), dtype=np.float32),
        g1=np.ones(256, np.float32),
        Wqkv=(rng.standard_normal((1536, 256), dtype=np.float32) * 256 ** -0.5),
        Wout=(rng.standard_normal((256, 512), dtype=np.float32) * 512 ** -0.5),
        bout=np.zeros(256, np.float32),
        g2=np.ones(256, np.float32),
    )
    out = kernel(**inputs)
    print("out", out.shape, out.dtype, np.abs(out).max())


# revision 27
# speedup vs baseline: 1.1652x; 1.1278x over previous
"""LinearAttention kernel for Trainium2, 8 NeuronCores, data-parallel over batch.

Reference (per batch, c=256 channels, n=4096 tokens):
  xn   = x / ||x||_c * g1 * 16
  qkv  = Wqkv @ xn            (q,k,v each [512, n])
  q    = softmax_d(q) / 8     (softmax over d=64 within each of 8 heads)
  k    = softmax_n(k)
  ctx_h = k_h @ v_h^T
  out  = Wout @ concat_h(ctx_h^T @ q_h) + bout
  out  = out / ||out||_c * g2 * 16

Sharding: 16 batches -> 8 cores x 2 batches. No collectives.

v4 design notes:
 - Projections in fp32r (1 cyc/row): fp8 in the value path costs ~4-6%
   relative error (random-sign contractions keep per-element quantization
   noise), blowing the 2e-2 budget.  fp8+DoubleRow only for the x^2
   channel-sum (positive sum, error averages).
 - Act uses only the natural_log_exp table set (preloaded explicitly
   once): rsqrt(s) = exp(-0.5*ln(s)) for both rms norms, exact exp for
   q and k, Square for the output norm. Zero table reloads.
 - Engines are in-order, so program order defines the pipeline: the two
   batches are tile-interleaved so both PSUM context banks accumulate
   concurrently and every engine sees work from the other batch while
   one batch waits on a dependency.
 - Streaming per-tile DMAs (in and out) instead of whole-batch buffers:
   loads never wait, stores trail each tile, and the SP sequencer is
   never head-of-line blocked on a cross-batch dependency.
 - Weight transpose/fold (g1 into Wqkv cols) on host; no device prologue.
 - Output stored bf16, upcast on host.
"""

import numpy as np

import concourse.bass as bass
import concourse.tile as tile
from concourse import bacc, mybir
from concourse.bass_utils import run_bass_kernel_spmd

F32 = mybir.dt.float32
F32R = mybir.dt.float32r
BF16 = mybir.dt.bfloat16
F8 = mybir.dt.float8e4
AF = mybir.ActivationFunctionType
OP = mybir.AluOpType
DR = mybir.MatmulPerfMode.DoubleRow

B = 16          # total batches
BL = 2          # batches per core
C = 256         # in channels
HID = 512       # heads * dim_head
HEADS = 8
DH = 64
N = 4096        # tokens
TN = 512        # token tile
NT = N // TN    # 8 tiles per batch
NB = TN // 128  # 4 128-token blocks per tile

ACT_TABLE_LN_EXP = 6  # index of natural_log_exp_and_others in act_func_sets


def build_kernel(with_bout: bool):
    nc = bacc.Bacc("TRN2", target_bir_lowering=False, debug=False, num_devices=8)

    x_d = nc.dram_tensor("x", [BL, C, N], BF16, kind="ExternalInput").ap()
    wq_d = nc.dram_tensor("wqkvT", [128, 2, 3 * HID], F32R, kind="ExternalInput").ap()
    wo_d = nc.dram_tensor("woutTb", [64, HEADS, C], F32R, kind="ExternalInput").ap()
    g2_d = nc.dram_tensor("g2c", [128, 2], F32, kind="ExternalInput").ap()
    if with_bout:
        bo_d = nc.dram_tensor("boutc", [128, 2], F32, kind="ExternalInput").ap()
    o_d = nc.dram_tensor("out", [BL, C, N], BF16, kind="ExternalOutput").ap()

    xv = x_d.rearrange("b (cb p) n -> b p cb n", cb=2)
    ov = o_d.rearrange("b (cb p) n -> b p cb n", cb=2)

    with tile.TileContext(nc) as tc:
        with (
            tc.tile_pool(name="const", bufs=1) as const,
            tc.tile_pool(name="big", bufs=1) as big,
            tc.tile_pool(name="work", bufs=1) as work,
            tc.tile_pool(name="ps", bufs=1, space="PSUM") as ps,
        ):
            # one activation-table load for the whole program
            nc.scalar.add_instruction(mybir.InstLoadActFuncSet(
                name=nc.get_next_instruction_name(), ins=[], outs=[],
                act_func_set_id=ACT_TABLE_LN_EXP))

            # ---- constants / weights (DMA straight into SBUF) ----
            wqkvT = const.tile([128, 2, 3 * HID], F32R)
            nc.sync.dma_start(out=wqkvT, in_=wq_d)
            woutTb = const.tile([64, HEADS, C], F32R)
            nc.sync.dma_start(out=woutTb, in_=wo_d)
            g2c = const.tile([128, 2], F32)
            nc.sync.dma_start(out=g2c, in_=g2_d)
            if with_bout:
                boutc = const.tile([128, 2], F32)
                nc.sync.dma_start(out=boutc, in_=bo_d)

            ones8 = const.tile([128, 2, 128], F8)
            nc.gpsimd.memset(ones8, 1.0)
            onesb = const.tile([128, 128], BF16)
            nc.gpsimd.memset(onesb, 1.0)
            bdb = const.tile([128, 128], BF16)
            nc.gpsimd.memset(bdb, 0.0)
            nc.gpsimd.memset(bdb[0:64, 0:64], 1.0)
            nc.gpsimd.memset(bdb[64:128, 64:128], 1.0)
            sclb = const.tile([1, 2], F32)
            nc.gpsimd.memset(sclb, 0.125)  # attention scale 1/8, via kdinv transpose

            # ---- per-batch persistent tensors ----
            q_sm = [None] * BL
            ctx_t = [None] * BL
            for bl in range(BL):
                q_sm[bl] = big.tile([128, 4, N], BF16, tag="qsm", bufs=2,
                                    name=f"qsm{bl}")
                ctx_t[bl] = ps.tile([128, 512], F32, tag="ctx", bufs=2,
                                    name=f"ctx{bl}")
                nc.vector.memset(ctx_t[bl], 0.0)

            # ====== batch epilogue: W2 = (Wout @ ctx^T / kden / 8)^T ======
            w2T = [None] * BL

            def epilogue(bl):
                kdinv = work.tile([1, 512], F32, tag="kdi", bufs=2)
                nc.vector.reciprocal(out=kdinv, in_=ctx_t[bl][64:65, :])
                ctx_sb = work.tile([64, 512], F32R, tag="ctxsb", bufs=2)
                nc.vector.tensor_copy(out=ctx_sb, in_=ctx_t[bl][0:64, :])
                pkd = ps.tile([128, 512], F32, tag="kv", bufs=2)
                for h in range(HEADS):
                    nc.tensor.matmul(
                        pkd[0:64, 2 * h:2 * h + 2],
                        kdinv[0:1, h * 64:(h + 1) * 64],
                        sclb, start=True, stop=True,
                    )
                kdcol = work.tile([64, HEADS, 1], F32, tag="kdcol", bufs=2)
                pkd_v = pkd[0:64, 0:16].rearrange("p (h t) -> p h t", t=2)
                nc.vector.tensor_copy(out=kdcol, in_=pkd_v[:, :, 0:1])
                w2T[bl] = work.tile([128, 4, 256], BF16, tag="w2T", bufs=2,
                                    name=f"w2T{bl}")
                for hh in range(4):
                    pw2 = ps.tile([64, 2, 256], F32, tag="pq", bufs=2)
                    for i in range(2):
                        h = hh * 2 + i
                        nc.tensor.matmul(
                            pw2[:, i, :],
                            ctx_sb[:, h * 64:(h + 1) * 64],
                            woutTb[:, h, :],
                            start=True, stop=True,
                        )
                    for i in range(2):
                        h = hh * 2 + i
                        nc.vector.tensor_scalar_mul(
                            out=w2T[bl][(h % 2) * 64:(h % 2) * 64 + 64, h // 2, :],
                            in0=pw2[:, i, :],
                            scalar1=kdcol[:, h, :],
                        )

            # ========== stage A: norm-prefix software-pipelined 1 tile ahead ==
            def prefix(j, bl):
                t0 = j * TN
                xin = work.tile([128, 2, TN], BF16, tag="xin", bufs=5)
                nc.sync.dma_start(out=xin, in_=xv[bl, :, :, t0:t0 + TN])
                # channel sum-of-squares -> 16/||x|| via exp(-0.5*ln(.))
                x2 = work.tile([128, 2, TN], F8, tag="x2", bufs=3)
                nc.gpsimd.tensor_mul(x2, xin, xin)
                ssq = ps.tile([128, 512], F32, tag="pq", bufs=2)
                nc.tensor.matmul(ssq, ones8, x2, start=True, stop=True,
                                 perf_mode=DR)
                lns = work.tile([128, TN], BF16, tag="lns", bufs=3)
                nc.scalar.activation(out=lns, in_=ssq, func=AF.Ln,
                                     scale=1.0 / 256.0)
                sinv = work.tile([128, TN], F32, tag="sinv", bufs=3)
                nc.scalar.activation(out=sinv, in_=lns, func=AF.Exp,
                                     scale=-0.5)
                xn = work.tile([128, 2, TN], F32R, tag="xn", bufs=5)
                for cb in range(2):
                    nc.gpsimd.tensor_mul(xn[:, cb, :], xin[:, cb, :], sinv)
                return xn

            def body_a(j, bl, xn):
                t0 = j * TN
                # q = Wq @ xn, fp32r (per-ob 1-bank PSUM tiles)
                eq = work.tile([128, 4, TN], BF16, tag="eq", bufs=3)
                qdi = work.tile([128, 4, TN], BF16, tag="qdi", bufs=3)
                for ob in range(4):
                    pq = ps.tile([128, 512], F32, tag="pq", bufs=2)
                    for cb in range(2):
                        nc.tensor.matmul(
                            pq,
                            wqkvT[:, cb, ob * 128:(ob + 1) * 128],
                            xn[:, cb, :],
                            start=(cb == 0), stop=(cb == 1),
                        )
                    nc.scalar.activation(out=eq[:, ob, :], in_=pq, func=AF.Exp)

                def kv_block(nb):
                    pkv = ps.tile([128, 1024], F32, tag="kv", bufs=2)
                    for half in range(2):
                        for cb in range(2):
                            nc.tensor.matmul(
                                pkv[:, half * 512:(half + 1) * 512],
                                xn[:, cb, nb * 128:(nb + 1) * 128],
                                wqkvT[:, cb, HID + half * 512:
                                      HID + (half + 1) * 512],
                                start=(cb == 0), stop=(cb == 1),
                            )
                    ek = work.tile([128, HEADS, DH], BF16, tag="ek", bufs=6)
                    nc.scalar.activation(
                        out=ek.rearrange("p h d -> p (h d)"),
                        in_=pkv[:, 0:512], func=AF.Exp)
                    vt = work.tile([128, HEADS, 65], BF16, tag="vt", bufs=6)
                    nc.gpsimd.memset(vt[:, :, 64:65], 1.0)
                    vsrc = pkv[:, 512:1024].rearrange("p (h e) -> p h e", h=8)
                    nc.vector.tensor_copy(out=vt[:, :, 0:64], in_=vsrc)
                    return ek, vt

                def ctx_block(nb, ekvt):
                    ek, vt = ekvt
                    gnb = j * NB + nb
                    for h in range(HEADS):
                        nc.tensor.matmul(
                            ctx_t[bl][0:65, h * DH:(h + 1) * DH],
                            vt[:, h, :],
                            ek[:, h, :],
                            start=False, stop=(gnb == N // 128 - 1),
                            skip_group_check=True,
                        )

                kv01 = [kv_block(0), kv_block(1)]
                # q softmax denominator while kv 2/3 project
                for ob in range(4):
                    pd = ps.tile([128, 512], F32, tag="pq", bufs=2)
                    nc.tensor.matmul(pd, bdb, eq[:, ob, :],
                                     start=True, stop=True)
                    with nc.allow_low_precision(reason="softmax recip bf16"):
                        nc.vector.reciprocal(out=qdi[:, ob, :], in_=pd)
                ctx_block(0, kv01[0])
                kv23 = [kv_block(2), kv_block(3)]
                nc.vector.tensor_mul(q_sm[bl][:, :, t0:t0 + TN], eq, qdi)
                ctx_block(1, kv01[1])
                ctx_block(2, kv23[0])
                ctx_block(3, kv23[1])

            xns = {}
            for bl in range(BL):
                xns[(0, bl)] = prefix(0, bl)
            for j in range(NT):
                for bl in range(BL):
                    if j + 1 < NT:
                        xns[(j + 1, bl)] = prefix(j + 1, bl)
                    body_a(j, bl, xns.pop((j, bl)))
                    if j == NT - 1:
                        epilogue(bl)

            # ================= stage B (tile-interleaved batches) ==========
            po_n = [0]

            def po_block(j, bl):
                t0 = j * TN
                pos = []
                for ob in range(2):
                    po = ps.tile([128, 512], F32, tag="pq", bufs=2)
                    pos.append(po)
                    for kb in range(4):
                        nc.tensor.matmul(
                            po,
                            w2T[bl][:, kb, ob * 128:(ob + 1) * 128],
                            q_sm[bl][:, kb, t0:t0 + TN],
                            start=(kb == 0), stop=(kb == 3),
                        )
                return pos

            def tail_b(j, bl, pos):
                t0 = j * TN
                if with_bout:
                    yb = work.tile([128, 2, TN], F32, tag="yb", bufs=2)
                    for cb in range(2):
                        nc.vector.tensor_scalar_add(
                            out=yb[:, cb, :], in0=pos[cb],
                            scalar1=boutc[:, cb:cb + 1])
                    ysrcs = [yb[:, 0, :], yb[:, 1, :]]
                else:
                    ysrcs = pos
                y2 = work.tile([128, 2, TN], BF16, tag="y2", bufs=3)
                for cb in range(2):
                    nc.scalar.activation(out=y2[:, cb, :], in_=ysrcs[cb],
                                         func=AF.Square)
                ssqo = ps.tile([128, 512], F32, tag="ctx", bufs=2)
                for cb in range(2):
                    nc.tensor.matmul(ssqo, onesb, y2[:, cb, :],
                                     start=(cb == 0), stop=(cb == 1))
                lno = work.tile([128, TN], F32, tag="lno", bufs=3)
                nc.scalar.activation(out=lno, in_=ssqo, func=AF.Ln,
                                     scale=1.0 / 256.0)
                rgo = work.tile([128, TN], F32, tag="rgo", bufs=3)
                nc.scalar.activation(out=rgo, in_=lno, func=AF.Exp,
                                     scale=-0.5)
                outt = work.tile([128, 2, TN], BF16, tag="outt", bufs=6)
                for cb in range(2):
                    nc.vector.scalar_tensor_tensor(
                        out=outt[:, cb, :],
                        in0=ysrcs[cb],
                        scalar=g2c[:, cb:cb + 1],
                        in1=rgo,
                        op0=OP.mult, op1=OP.mult,
                    )
                nc.sync.dma_start(out=ov[bl, :, :, t0:t0 + TN], in_=outt)

            for j in range(NT):
                pos = [po_block(j, bl) for bl in range(BL)]
                for bl in range(BL):
                    tail_b(j, bl, pos[bl])

    nc.finalize()
    return nc


_NC_CACHE = {}


def kernel(x, g1, Wqkv, Wout, bout, g2):
    x = np.ascontiguousarray(np.asarray(x, dtype=np.float32))
    g1 = np.asarray(g1, dtype=np.float32)
    Wqkv = np.asarray(Wqkv, dtype=np.float32)
    Wout = np.asarray(Wout, dtype=np.float32)
    bout = np.asarray(bout, dtype=np.float32)
    g2 = np.asarray(g2, dtype=np.float32)

    b, c, H, W = x.shape
    xr = x.reshape(b, c, H * W)

    bf = mybir.dt.np(BF16)
    # WqkvT [c, 3H] with g1 folded per channel, [p, cb, 3H] layout
    wqkvT = np.ascontiguousarray(
        (Wqkv.T * g1[:, None]).reshape(2, 128, 3 * HID).transpose(1, 0, 2)
    ).astype(np.float32)
    woutTb = np.ascontiguousarray(
        Wout.reshape(C, HEADS, DH).transpose(2, 1, 0)).astype(np.float32)
    g2c = np.ascontiguousarray(g2.reshape(2, 128).T)
    with_bout = bool(np.any(bout))

    if with_bout not in _NC_CACHE:
        _NC_CACHE[with_bout] = build_kernel(with_bout)
    nc = _NC_CACHE[with_bout]

    in_maps = []
    for core in range(8):
        m = {
            "x": np.ascontiguousarray(xr[core * BL:(core + 1) * BL].astype(bf)),
            "wqkvT": wqkvT, "woutTb": woutTb, "g2c": g2c,
        }
        if with_bout:
            m["boutc"] = np.ascontiguousarray(bout.reshape(2, 128).T)
        in_maps.append(m)
    res = run_bass_kernel_spmd(nc, in_maps, core_ids=list(range(8)))
    out = np.concatenate(
        [np.asarray(m["out"]).astype(np.float32) for m in res.results], axis=0)
    return out.reshape(b, c, H, W)


if __name__ == "__main__":
    rng = np.random.default_rng(0)
    inputs = dict(
        x=rng.standard_normal((16, 256, 64, 64), dtype=np.float32),
        g1=np.ones(256, np.float32),
        Wqkv=(rng.standard_normal((1536, 256), dtype=np.float32) * 256 ** -0.5),
        Wout=(rng.standard_normal((256, 512), dtype=np.float32) * 512 ** -0.5),
        bout=np.zeros(256, np.float32),
        g2=np.ones(256, np.float32),
    )
    out = kernel(**inputs)
    print("out", out.shape, out.dtype, np.abs(out).max())


# revision 28
# speedup vs baseline: 1.3123x; 1.1263x over previous
"""LinearAttention kernel for Trainium2, 8 NeuronCores, data-parallel over batch.

Reference (per batch, c=256 channels, n=4096 tokens):
  xn   = x / ||x||_c * g1 * 16
  qkv  = Wqkv @ xn            (q,k,v each [512, n])
  q    = softmax_d(q) / 8     (softmax over d=64 within each of 8 heads)
  k    = softmax_n(k)
  ctx_h = k_h @ v_h^T
  out  = Wout @ concat_h(ctx_h^T @ q_h) + bout
  out  = out / ||out||_c * g2 * 16

Sharding: 16 batches -> 8 cores x 2 batches. No collectives.

v4 design notes:
 - Projections in fp32r (1 cyc/row): fp8 in the value path costs ~4-6%
   relative error (random-sign contractions keep per-element quantization
   noise), blowing the 2e-2 budget.  fp8+DoubleRow only for the x^2
   channel-sum (positive sum, error averages).
 - Act uses only the natural_log_exp table set (preloaded explicitly
   once): rsqrt(s) = exp(-0.5*ln(s)) for both rms norms, exact exp for
   q and k, Square for the output norm. Zero table reloads.
 - Engines are in-order, so program order defines the pipeline: the two
   batches are tile-interleaved so both PSUM context banks accumulate
   concurrently and every engine sees work from the other batch while
   one batch waits on a dependency.
 - Streaming per-tile DMAs (in and out) instead of whole-batch buffers:
   loads never wait, stores trail each tile, and the SP sequencer is
   never head-of-line blocked on a cross-batch dependency.
 - Weight transpose/fold (g1 into Wqkv cols) on host; no device prologue.
 - Output stored bf16, upcast on host.
"""

import numpy as np

import concourse.bass as bass
import concourse.tile as tile
from concourse import bacc, mybir
from concourse.bass_utils import run_bass_kernel_spmd

F32 = mybir.dt.float32
F32R = mybir.dt.float32r
BF16 = mybir.dt.bfloat16
F8 = mybir.dt.float8e4
AF = mybir.ActivationFunctionType
OP = mybir.AluOpType
DR = mybir.MatmulPerfMode.DoubleRow

B = 16          # total batches
BL = 2          # batches per core
C = 256         # in channels
HID = 512       # heads * dim_head
HEADS = 8
DH = 64
N = 4096        # tokens
TN = 512        # token tile
NT = N // TN    # 8 tiles per batch
NB = TN // 128  # 4 128-token blocks per tile

ACT_TABLE_LN_EXP = 6  # index of natural_log_exp_and_others in act_func_sets


def build_kernel(with_bout: bool):
    nc = bacc.Bacc("TRN2", target_bir_lowering=False, debug=False, num_devices=8)

    x_d = nc.dram_tensor("x", [BL, C, N], BF16, kind="ExternalInput").ap()
    wq_d = nc.dram_tensor("wqkvT", [128, 2, 3 * HID], F32R, kind="ExternalInput").ap()
    wo_d = nc.dram_tensor("woutTb", [64, HEADS, C], F32R, kind="ExternalInput").ap()
    g2_d = nc.dram_tensor("g2c", [128, 2], F32, kind="ExternalInput").ap()
    if with_bout:
        bo_d = nc.dram_tensor("boutc", [128, 2], F32, kind="ExternalInput").ap()
    o_d = nc.dram_tensor("out", [BL, C, N], BF16, kind="ExternalOutput").ap()

    xv = x_d.rearrange("b (cb p) n -> b p cb n", cb=2)
    ov = o_d.rearrange("b (cb p) n -> b p cb n", cb=2)

    with tile.TileContext(nc) as tc:
        with (
            tc.tile_pool(name="const", bufs=1) as const,
            tc.tile_pool(name="big", bufs=1) as big,
            tc.tile_pool(name="work", bufs=1) as work,
            tc.tile_pool(name="ps", bufs=1, space="PSUM") as ps,
        ):
            # one activation-table load for the whole program
            nc.scalar.add_instruction(mybir.InstLoadActFuncSet(
                name=nc.get_next_instruction_name(), ins=[], outs=[],
                act_func_set_id=ACT_TABLE_LN_EXP))

            # ---- constants / weights (DMA straight into SBUF) ----
            wqkvT = const.tile([128, 2, 3 * HID], F32R)
            nc.sync.dma_start(out=wqkvT, in_=wq_d)
            woutTb = const.tile([64, HEADS, C], F32R)
            nc.sync.dma_start(out=woutTb, in_=wo_d)
            g2c = const.tile([128, 2], F32)
            nc.sync.dma_start(out=g2c, in_=g2_d)
            if with_bout:
                boutc = const.tile([128, 2], F32)
                nc.sync.dma_start(out=boutc, in_=bo_d)

            ones8 = const.tile([128, 2, 128], F8)
            nc.gpsimd.memset(ones8, 1.0)
            onesb = const.tile([128, 128], BF16)
            nc.gpsimd.memset(onesb, 1.0)
            bdb = const.tile([128, 128], BF16)
            nc.gpsimd.memset(bdb, 0.0)
            nc.gpsimd.memset(bdb[0:64, 0:64], 1.0)
            nc.gpsimd.memset(bdb[64:128, 64:128], 1.0)
            sclb = const.tile([1, 2], F32)
            nc.gpsimd.memset(sclb, 0.125)  # attention scale 1/8, via kdinv transpose

            # ---- per-batch persistent tensors ----
            q_sm = [None] * BL
            ctx_t = [None] * BL
            for bl in range(BL):
                q_sm[bl] = big.tile([128, 4, N], BF16, tag="qsm", bufs=2,
                                    name=f"qsm{bl}")
                ctx_t[bl] = ps.tile([128, 512], F32, tag="ctx", bufs=2,
                                    name=f"ctx{bl}")
                nc.vector.memset(ctx_t[bl], 0.0)

            # ====== batch epilogue: W2 = (Wout @ ctx^T / kden / 8)^T ======
            w2T = [None] * BL

            def epilogue(bl):
                kdinv = work.tile([1, 512], F32, tag="kdi", bufs=2)
                nc.vector.reciprocal(out=kdinv, in_=ctx_t[bl][64:65, :])
                ctx_sb = work.tile([64, 512], F32R, tag="ctxsb", bufs=2)
                nc.vector.tensor_copy(out=ctx_sb, in_=ctx_t[bl][0:64, :])
                pkd = ps.tile([128, 512], F32, tag="kv", bufs=2)
                for h in range(HEADS):
                    nc.tensor.matmul(
                        pkd[0:64, 2 * h:2 * h + 2],
                        kdinv[0:1, h * 64:(h + 1) * 64],
                        sclb, start=True, stop=True,
                    )
                kdcol = work.tile([64, HEADS, 1], F32, tag="kdcol", bufs=2)
                pkd_v = pkd[0:64, 0:16].rearrange("p (h t) -> p h t", t=2)
                nc.vector.tensor_copy(out=kdcol, in_=pkd_v[:, :, 0:1])
                w2T[bl] = work.tile([128, 4, 256], BF16, tag="w2T", bufs=2,
                                    name=f"w2T{bl}")
                for hh in range(4):
                    pw2 = ps.tile([64, 2, 256], F32, tag="pq", bufs=2)
                    for i in range(2):
                        h = hh * 2 + i
                        nc.tensor.matmul(
                            pw2[:, i, :],
                            ctx_sb[:, h * 64:(h + 1) * 64],
                            woutTb[:, h, :],
                            start=True, stop=True,
                        )
                    for i in range(2):
                        h = hh * 2 + i
                        nc.vector.tensor_scalar_mul(
                            out=w2T[bl][(h % 2) * 64:(h % 2) * 64 + 64, h // 2, :],
                            in0=pw2[:, i, :],
                            scalar1=kdcol[:, h, :],
                        )

            # ========== stage A: norm-prefix software-pipelined 1 tile ahead ==
            def prefix(j, bl):
                t0 = j * TN
                xin = work.tile([128, 2, TN], BF16, tag="xin", bufs=5)
                nc.sync.dma_start(out=xin, in_=xv[bl, :, :, t0:t0 + TN])
                # channel sum-of-squares -> 16/||x|| via exp(-0.5*ln(.))
                x2 = work.tile([128, 2, TN], F8, tag="x2", bufs=3)
                nc.gpsimd.tensor_mul(x2, xin, xin)
                ssq = ps.tile([128, 512], F32, tag="pq", bufs=2)
                nc.tensor.matmul(ssq, ones8, x2, start=True, stop=True,
                                 perf_mode=DR)
                lns = work.tile([128, TN], BF16, tag="lns", bufs=3)
                nc.scalar.activation(out=lns, in_=ssq, func=AF.Ln,
                                     scale=1.0 / 256.0)
                sinv = work.tile([128, TN], F32, tag="sinv", bufs=3)
                nc.scalar.activation(out=sinv, in_=lns, func=AF.Exp,
                                     scale=-0.5)
                xn = work.tile([128, 2, TN], F32R, tag="xn", bufs=5)
                for cb in range(2):
                    nc.gpsimd.tensor_mul(xn[:, cb, :], xin[:, cb, :], sinv)
                return xn

            def body_a(j, bl, xn):
                t0 = j * TN
                # q = Wq @ xn, fp32r (per-ob 1-bank PSUM tiles)
                eq = work.tile([128, 4, TN], BF16, tag="eq", bufs=3)
                qdi = work.tile([128, 4, TN], BF16, tag="qdi", bufs=3)
                for ob in range(4):
                    pq = ps.tile([128, 512], F32, tag="pq", bufs=2)
                    for cb in range(2):
                        nc.tensor.matmul(
                            pq,
                            wqkvT[:, cb, ob * 128:(ob + 1) * 128],
                            xn[:, cb, :],
                            start=(cb == 0), stop=(cb == 1),
                        )
                    nc.scalar.activation(out=eq[:, ob, :], in_=pq, func=AF.Exp)

                def kv_block(nb):
                    pkv = ps.tile([128, 1024], F32, tag="kv", bufs=2)
                    for half in range(2):
                        for cb in range(2):
                            nc.tensor.matmul(
                                pkv[:, half * 512:(half + 1) * 512],
                                xn[:, cb, nb * 128:(nb + 1) * 128],
                                wqkvT[:, cb, HID + half * 512:
                                      HID + (half + 1) * 512],
                                start=(cb == 0), stop=(cb == 1),
                            )
                    ek = work.tile([128, HEADS, DH], BF16, tag="ek", bufs=6)
                    nc.scalar.activation(
                        out=ek.rearrange("p h d -> p (h d)"),
                        in_=pkv[:, 0:512], func=AF.Exp)
                    vt = work.tile([128, HEADS, 65], BF16, tag="vt", bufs=6)
                    nc.gpsimd.memset(vt[:, :, 64:65], 1.0)
                    vsrc = pkv[:, 512:1024].rearrange("p (h e) -> p h e", h=8)
                    nc.vector.tensor_copy(out=vt[:, :, 0:64], in_=vsrc)
                    return ek, vt

                def ctx_block(nb, ekvt):
                    ek, vt = ekvt
                    gnb = j * NB + nb
                    for h in range(HEADS):
                        nc.tensor.matmul(
                            ctx_t[bl][0:65, h * DH:(h + 1) * DH],
                            vt[:, h, :],
                            ek[:, h, :],
                            start=False, stop=(gnb == N // 128 - 1),
                            skip_group_check=True,
                        )

                kv01 = [kv_block(0), kv_block(1)]
                # q softmax denominator while kv 2/3 project
                for ob in range(4):
                    pd = ps.tile([128, 512], F32, tag="pq", bufs=2)
                    nc.tensor.matmul(pd, bdb, eq[:, ob, :],
                                     start=True, stop=True)
                    with nc.allow_low_precision(reason="softmax recip bf16"):
                        nc.vector.reciprocal(out=qdi[:, ob, :], in_=pd)
                ctx_block(0, kv01[0])
                kv23 = [kv_block(2), kv_block(3)]
                nc.vector.tensor_mul(q_sm[bl][:, :, t0:t0 + TN], eq, qdi)
                ctx_block(1, kv01[1])
                ctx_block(2, kv23[0])
                ctx_block(3, kv23[1])

            xns = {}
            for bl in range(BL):
                xns[(0, bl)] = prefix(0, bl)
            for j in range(NT):
                for bl in range(BL):
                    if j + 1 < NT:
                        xns[(j + 1, bl)] = prefix(j + 1, bl)
                    body_a(j, bl, xns.pop((j, bl)))
                    if j == NT - 1:
                        epilogue(bl)

            # ================= stage B (tile-interleaved batches) ==========
            po_n = [0]

            def po_block(j, bl):
                t0 = j * TN
                pos = []
                for ob in range(2):
                    if bl == 0:
                        po = ps.tile([128, 512], F32, tag="pq", bufs=2)
                    else:
                        pkv = ps.tile([128, 1024], F32, tag="kv", bufs=2)
                        po = pkv[:, 0:512]
                    pos.append(po)
                    for kb in range(4):
                        nc.tensor.matmul(
                            po,
                            w2T[bl][:, kb, ob * 128:(ob + 1) * 128],
                            q_sm[bl][:, kb, t0:t0 + TN],
                            start=(kb == 0), stop=(kb == 3),
                        )
                return pos

            def tail_b(j, bl, pos):
                t0 = j * TN
                if with_bout:
                    yb = work.tile([128, 2, TN], F32, tag="yb", bufs=2)
                    for cb in range(2):
                        nc.vector.tensor_scalar_add(
                            out=yb[:, cb, :], in0=pos[cb],
                            scalar1=boutc[:, cb:cb + 1])
                    ysrcs = [yb[:, 0, :], yb[:, 1, :]]
                else:
                    ysrcs = pos
                y2 = work.tile([128, 2, TN], BF16, tag="y2", bufs=3)
                for cb in range(2):
                    nc.scalar.activation(out=y2[:, cb, :], in_=ysrcs[cb],
                                         func=AF.Square)
                ssqo = ps.tile([128, 512], F32, tag="ctx", bufs=2)
                for cb in range(2):
                    nc.tensor.matmul(ssqo, onesb, y2[:, cb, :],
                                     start=(cb == 0), stop=(cb == 1))
                lno = work.tile([128, TN], F32, tag="lno", bufs=3)
                nc.scalar.activation(out=lno, in_=ssqo, func=AF.Ln,
                                     scale=1.0 / 256.0)
                rgo = work.tile([128, TN], F32, tag="rgo", bufs=3)
                nc.scalar.activation(out=rgo, in_=lno, func=AF.Exp,
                                     scale=-0.5)
                outt = work.tile([128, 2, TN], BF16, tag="outt", bufs=6)
                for cb in range(2):
                    nc.vector.scalar_tensor_tensor(
                        out=outt[:, cb, :],
                        in0=ysrcs[cb],
                        scalar=g2c[:, cb:cb + 1],
                        in1=rgo,
                        op0=OP.mult, op1=OP.mult,
                    )
                nc.sync.dma_start(out=ov[bl, :, :, t0:t0 + TN], in_=outt)

            for j in range(NT):
                pos = [po_block(j, bl) for bl in range(BL)]
                for bl in range(BL):
                    tail_b(j, bl, pos[bl])

    nc.finalize()
    return nc


_NC_CACHE = {}


def kernel(x, g1, Wqkv, Wout, bout, g2):
    x = np.ascontiguousarray(np.asarray(x, dtype=np.float32))
    g1 = np.asarray(g1, dtype=np.float32)
    Wqkv = np.asarray(Wqkv, dtype=np.float32)
    Wout = np.asarray(Wout, dtype=np.float32)
    bout = np.asarray(bout, dtype=np.float32)
    g2 = np.asarray(g2, dtype=np.float32)

    b, c, H, W = x.shape
    xr = x.reshape(b, c, H * W)

    bf = mybir.dt.np(BF16)
    # WqkvT [c, 3H] with g1 folded per channel, [p, cb, 3H] layout
    wqkvT = np.ascontiguousarray(
        (Wqkv.T * g1[:, None]).reshape(2, 128, 3 * HID).transpose(1, 0, 2)
    ).astype(np.float32)
    woutTb = np.ascontiguousarray(
        Wout.reshape(C, HEADS, DH).transpose(2, 1, 0)).astype(np.float32)
    g2c = np.ascontiguousarray(g2.reshape(2, 128).T)
    with_bout = bool(np.any(bout))

    if with_bout not in _NC_CACHE:
        _NC_CACHE[with_bout] = build_kernel(with_bout)
    nc = _NC_CACHE[with_bout]

    in_maps = []
    for core in range(8):
        m = {
            "x": np.ascontiguousarray(xr[core * BL:(core + 1) * BL].astype(bf)),
            "wqkvT": wqkvT, "woutTb": woutTb, "g2c": g2c,
        }
        if with_bout:
            m["boutc"] = np.ascontiguousarray(bout.reshape(2, 128).T)
        in_maps.append(m)
    res = run_bass_kernel_spmd(nc, in_maps, core_ids=list(range(8)))
    out = np.concatenate(
        [np.asarray(m["out"]).astype(np.float32) for m in res.results], axis=0)
    return out.reshape(b, c, H, W)


if __name__ == "__main__":
    rng = np.random.default_rng(0)
    inputs = dict(
        x=rng.standard_normal((16, 256, 64, 64), dtype=np.float32),
        g1=np.ones(256, np.float32),
        Wqkv=(rng.standard_normal((1536, 256), dtype=np.float32) * 256 ** -0.5),
        Wout=(rng.standard_normal((256, 512), dtype=np.float32) * 512 ** -0.5),
        bout=np.zeros(256, np.float32),
        g2=np.ones(256, np.float32),
    )
    out = kernel(**inputs)
    print("out", out.shape, out.dtype, np.abs(out).max())


# revision 29
# speedup vs baseline: 1.3250x; 1.0096x over previous
"""LinearAttention kernel for Trainium2, 8 NeuronCores, data-parallel over batch.

Reference (per batch, c=256 channels, n=4096 tokens):
  xn   = x / ||x||_c * g1 * 16
  qkv  = Wqkv @ xn            (q,k,v each [512, n])
  q    = softmax_d(q) / 8     (softmax over d=64 within each of 8 heads)
  k    = softmax_n(k)
  ctx_h = k_h @ v_h^T
  out  = Wout @ concat_h(ctx_h^T @ q_h) + bout
  out  = out / ||out||_c * g2 * 16

Sharding: 16 batches -> 8 cores x 2 batches. No collectives.

v4 design notes:
 - Projections in fp32r (1 cyc/row): fp8 in the value path costs ~4-6%
   relative error (random-sign contractions keep per-element quantization
   noise), blowing the 2e-2 budget.  fp8+DoubleRow only for the x^2
   channel-sum (positive sum, error averages).
 - Act uses only the natural_log_exp table set (preloaded explicitly
   once): rsqrt(s) = exp(-0.5*ln(s)) for both rms norms, exact exp for
   q and k, Square for the output norm. Zero table reloads.
 - Engines are in-order, so program order defines the pipeline: the two
   batches are tile-interleaved so both PSUM context banks accumulate
   concurrently and every engine sees work from the other batch while
   one batch waits on a dependency.
 - Streaming per-tile DMAs (in and out) instead of whole-batch buffers:
   loads never wait, stores trail each tile, and the SP sequencer is
   never head-of-line blocked on a cross-batch dependency.
 - Weight transpose/fold (g1 into Wqkv cols) on host; no device prologue.
 - Output stored bf16, upcast on host.
"""

import numpy as np

import concourse.bass as bass
import concourse.tile as tile
from concourse import bacc, mybir
from concourse.bass_utils import run_bass_kernel_spmd

F32 = mybir.dt.float32
F32R = mybir.dt.float32r
BF16 = mybir.dt.bfloat16
F8 = mybir.dt.float8e4
AF = mybir.ActivationFunctionType
OP = mybir.AluOpType
DR = mybir.MatmulPerfMode.DoubleRow

B = 16          # total batches
BL = 2          # batches per core
C = 256         # in channels
HID = 512       # heads * dim_head
HEADS = 8
DH = 64
N = 4096        # tokens
TN = 512        # token tile
NT = N // TN    # 8 tiles per batch
NB = TN // 128  # 4 128-token blocks per tile

ACT_TABLE_LN_EXP = 6  # index of natural_log_exp_and_others in act_func_sets


def build_kernel(with_bout: bool):
    nc = bacc.Bacc("TRN2", target_bir_lowering=False, debug=False, num_devices=8)

    x_d = nc.dram_tensor("x", [BL, C, N], BF16, kind="ExternalInput").ap()
    wq_d = nc.dram_tensor("wqkvT", [128, 2, 3 * HID], F32R, kind="ExternalInput").ap()
    wo_d = nc.dram_tensor("woutTb", [64, HEADS, C], F32R, kind="ExternalInput").ap()
    g2_d = nc.dram_tensor("g2c", [128, 2], F32, kind="ExternalInput").ap()
    if with_bout:
        bo_d = nc.dram_tensor("boutc", [128, 2], F32, kind="ExternalInput").ap()
    o_d = nc.dram_tensor("out", [BL, C, N], BF16, kind="ExternalOutput").ap()

    xv = x_d.rearrange("b (cb p) n -> b p cb n", cb=2)
    ov = o_d.rearrange("b (cb p) n -> b p cb n", cb=2)

    with tile.TileContext(nc) as tc:
        with (
            tc.tile_pool(name="const", bufs=1) as const,
            tc.tile_pool(name="big", bufs=1) as big,
            tc.tile_pool(name="work", bufs=1) as work,
            tc.tile_pool(name="ps", bufs=1, space="PSUM") as ps,
        ):
            # one activation-table load for the whole program
            nc.scalar.add_instruction(mybir.InstLoadActFuncSet(
                name=nc.get_next_instruction_name(), ins=[], outs=[],
                act_func_set_id=ACT_TABLE_LN_EXP))

            # ---- constants / weights (DMA straight into SBUF) ----
            wqkvT = const.tile([128, 2, 3 * HID], F32R)
            nc.sync.dma_start(out=wqkvT, in_=wq_d)
            woutTb = const.tile([64, HEADS, C], F32R)
            nc.sync.dma_start(out=woutTb, in_=wo_d)
            g2c = const.tile([128, 2], F32)
            nc.sync.dma_start(out=g2c, in_=g2_d)
            if with_bout:
                boutc = const.tile([128, 2], F32)
                nc.sync.dma_start(out=boutc, in_=bo_d)

            ones8 = const.tile([128, 2, 128], F8)
            nc.gpsimd.memset(ones8, 1.0)
            onesb = const.tile([128, 128], BF16)
            nc.gpsimd.memset(onesb, 1.0)
            bdb = const.tile([128, 128], BF16)
            nc.gpsimd.memset(bdb, 0.0)
            nc.gpsimd.memset(bdb[0:64, 0:64], 1.0)
            nc.gpsimd.memset(bdb[64:128, 64:128], 1.0)
            sclb = const.tile([1, 2], F32)
            nc.gpsimd.memset(sclb, 0.125)  # attention scale 1/8, via kdinv transpose

            # ---- per-batch persistent tensors ----
            q_sm = [None] * BL
            ctx_t = [None] * BL
            for bl in range(BL):
                q_sm[bl] = big.tile([128, 4, N], BF16, tag="qsm", bufs=2,
                                    name=f"qsm{bl}")
                ctx_t[bl] = ps.tile([128, 512], F32, tag="ctx", bufs=2,
                                    name=f"ctx{bl}")
                nc.vector.memset(ctx_t[bl], 0.0)

            # ====== batch epilogue: W2 = (Wout @ ctx^T / kden / 8)^T ======
            w2T = [None] * BL

            def epilogue(bl):
                kdinv = work.tile([1, 512], F32, tag="kdi", bufs=2)
                nc.vector.reciprocal(out=kdinv, in_=ctx_t[bl][64:65, :])
                ctx_sb = work.tile([64, 512], F32R, tag="ctxsb", bufs=2)
                nc.vector.tensor_copy(out=ctx_sb, in_=ctx_t[bl][0:64, :])
                pkd = ps.tile([128, 512], F32, tag="kv", bufs=2)
                for h in range(HEADS):
                    nc.tensor.matmul(
                        pkd[0:64, 2 * h:2 * h + 2],
                        kdinv[0:1, h * 64:(h + 1) * 64],
                        sclb, start=True, stop=True,
                    )
                kdcol = work.tile([64, HEADS, 1], F32, tag="kdcol", bufs=2)
                pkd_v = pkd[0:64, 0:16].rearrange("p (h t) -> p h t", t=2)
                nc.vector.tensor_copy(out=kdcol, in_=pkd_v[:, :, 0:1])
                w2T[bl] = work.tile([128, 4, 256], BF16, tag="w2T", bufs=2,
                                    name=f"w2T{bl}")
                for hh in range(4):
                    pw2 = ps.tile([64, 2, 256], F32, tag="pq", bufs=2)
                    for i in range(2):
                        h = hh * 2 + i
                        nc.tensor.matmul(
                            pw2[:, i, :],
                            ctx_sb[:, h * 64:(h + 1) * 64],
                            woutTb[:, h, :],
                            start=True, stop=True,
                        )
                    for i in range(2):
                        h = hh * 2 + i
                        nc.vector.tensor_scalar_mul(
                            out=w2T[bl][(h % 2) * 64:(h % 2) * 64 + 64, h // 2, :],
                            in0=pw2[:, i, :],
                            scalar1=kdcol[:, h, :],
                        )

            # ========== stage A: norm-prefix software-pipelined 1 tile ahead ==
            def prefix(j, bl):
                t0 = j * TN
                xin = work.tile([128, 2, TN], BF16, tag="xin", bufs=5)
                nc.sync.dma_start(out=xin, in_=xv[bl, :, :, t0:t0 + TN])
                # channel sum-of-squares -> 16/||x|| via exp(-0.5*ln(.))
                x2 = work.tile([128, 2, TN], F8, tag="x2", bufs=3)
                nc.gpsimd.tensor_mul(x2, xin, xin)
                ssq = ps.tile([128, 512], F32, tag="pq", bufs=2)
                nc.tensor.matmul(ssq, ones8, x2, start=True, stop=True,
                                 perf_mode=DR)
                lns = work.tile([128, TN], BF16, tag="lns", bufs=3)
                nc.scalar.activation(out=lns, in_=ssq, func=AF.Ln,
                                     scale=1.0 / 256.0)
                sinv = work.tile([128, TN], F32, tag="sinv", bufs=3)
                nc.scalar.activation(out=sinv, in_=lns, func=AF.Exp,
                                     scale=-0.5)
                xn = work.tile([128, 2, TN], F32R, tag="xn", bufs=5)
                for cb in range(2):
                    nc.gpsimd.tensor_mul(xn[:, cb, :], xin[:, cb, :], sinv)
                return xn

            def body_a(j, bl, xn):
                t0 = j * TN
                # q = Wq @ xn, fp32r (per-ob 1-bank PSUM tiles)
                eq = work.tile([128, 4, TN], BF16, tag="eq", bufs=3)
                qdi = work.tile([128, 4, TN], BF16, tag="qdi", bufs=3)
                for ob in range(4):
                    pq = ps.tile([128, 512], F32, tag="pq", bufs=2)
                    for cb in range(2):
                        nc.tensor.matmul(
                            pq,
                            wqkvT[:, cb, ob * 128:(ob + 1) * 128],
                            xn[:, cb, :],
                            start=(cb == 0), stop=(cb == 1),
                        )
                    nc.scalar.activation(out=eq[:, ob, :], in_=pq, func=AF.Exp)

                def kv_block(nb):
                    pkv = ps.tile([128, 1024], F32, tag="kv", bufs=2)
                    for half in range(2):
                        for cb in range(2):
                            nc.tensor.matmul(
                                pkv[:, half * 512:(half + 1) * 512],
                                xn[:, cb, nb * 128:(nb + 1) * 128],
                                wqkvT[:, cb, HID + half * 512:
                                      HID + (half + 1) * 512],
                                start=(cb == 0), stop=(cb == 1),
                            )
                    ek = work.tile([128, HEADS, DH], BF16, tag="ek", bufs=6)
                    nc.scalar.activation(
                        out=ek.rearrange("p h d -> p (h d)"),
                        in_=pkv[:, 0:512], func=AF.Exp)
                    vt = work.tile([128, HEADS, 65], BF16, tag="vt", bufs=6)
                    nc.gpsimd.memset(vt[:, :, 64:65], 1.0)
                    vsrc = pkv[:, 512:1024].rearrange("p (h e) -> p h e", h=8)
                    nc.vector.tensor_copy(out=vt[:, :, 0:64], in_=vsrc)
                    return ek, vt

                def ctx_block(nb, ekvt):
                    ek, vt = ekvt
                    gnb = j * NB + nb
                    for h in range(HEADS):
                        nc.tensor.matmul(
                            ctx_t[bl][0:65, h * DH:(h + 1) * DH],
                            vt[:, h, :],
                            ek[:, h, :],
                            start=False, stop=(gnb == N // 128 - 1),
                            skip_group_check=True,
                        )

                kv01 = [kv_block(0), kv_block(1)]
                # q softmax denominator while kv 2/3 project
                for ob in range(4):
                    pd = ps.tile([128, 512], F32, tag="pq", bufs=2)
                    nc.tensor.matmul(pd, bdb, eq[:, ob, :],
                                     start=True, stop=True)
                    with nc.allow_low_precision(reason="softmax recip bf16"):
                        nc.vector.reciprocal(out=qdi[:, ob, :], in_=pd)
                ctx_block(0, kv01[0])
                kv23 = [kv_block(2), kv_block(3)]
                nc.vector.tensor_mul(q_sm[bl][:, :, t0:t0 + TN], eq, qdi)
                ctx_block(1, kv01[1])
                ctx_block(2, kv23[0])
                ctx_block(3, kv23[1])

            xns = {}
            for jj in range(2):
                for bl in range(BL):
                    xns[(jj, bl)] = prefix(jj, bl)
            for j in range(NT):
                for bl in range(BL):
                    if j + 2 < NT:
                        xns[(j + 2, bl)] = prefix(j + 2, bl)
                    body_a(j, bl, xns.pop((j, bl)))
                    if j == NT - 1:
                        epilogue(bl)

            # ================= stage B (tile-interleaved batches) ==========
            po_n = [0]

            def po_block(j, bl):
                t0 = j * TN
                pos = []
                for ob in range(2):
                    if bl == 0:
                        po = ps.tile([128, 512], F32, tag="pq", bufs=2)
                    else:
                        pkv = ps.tile([128, 1024], F32, tag="kv", bufs=2)
                        po = pkv[:, 0:512]
                    pos.append(po)
                    for kb in range(4):
                        nc.tensor.matmul(
                            po,
                            w2T[bl][:, kb, ob * 128:(ob + 1) * 128],
                            q_sm[bl][:, kb, t0:t0 + TN],
                            start=(kb == 0), stop=(kb == 3),
                        )
                return pos

            def tail_b(j, bl, pos):
                t0 = j * TN
                if with_bout:
                    yb = work.tile([128, 2, TN], F32, tag="yb", bufs=2)
                    for cb in range(2):
                        nc.vector.tensor_scalar_add(
                            out=yb[:, cb, :], in0=pos[cb],
                            scalar1=boutc[:, cb:cb + 1])
                    ysrcs = [yb[:, 0, :], yb[:, 1, :]]
                else:
                    ysrcs = pos
                y2 = work.tile([128, 2, TN], BF16, tag="y2", bufs=3)
                for cb in range(2):
                    nc.scalar.activation(out=y2[:, cb, :], in_=ysrcs[cb],
                                         func=AF.Square)
                ssqo = ps.tile([128, 512], F32, tag="ctx", bufs=2)
                for cb in range(2):
                    nc.tensor.matmul(ssqo, onesb, y2[:, cb, :],
                                     start=(cb == 0), stop=(cb == 1))
                lno = work.tile([128, TN], F32, tag="lno", bufs=3)
                nc.scalar.activation(out=lno, in_=ssqo, func=AF.Ln,
                                     scale=1.0 / 256.0)
                rgo = work.tile([128, TN], F32, tag="rgo", bufs=3)
                nc.scalar.activation(out=rgo, in_=lno, func=AF.Exp,
                                     scale=-0.5)
                outt = work.tile([128, 2, TN], BF16, tag="outt", bufs=6)
                for cb in range(2):
                    nc.vector.scalar_tensor_tensor(
                        out=outt[:, cb, :],
                        in0=ysrcs[cb],
                        scalar=g2c[:, cb:cb + 1],
                        in1=rgo,
                        op0=OP.mult, op1=OP.mult,
                    )
                nc.sync.dma_start(out=ov[bl, :, :, t0:t0 + TN], in_=outt)

            for j in range(NT):
                pos = [po_block(j, bl) for bl in range(BL)]
                for bl in range(BL):
                    tail_b(j, bl, pos[bl])

    nc.finalize()
    return nc


_NC_CACHE = {}


def kernel(x, g1, Wqkv, Wout, bout, g2):
    x = np.ascontiguousarray(np.asarray(x, dtype=np.float32))
    g1 = np.asarray(g1, dtype=np.float32)
    Wqkv = np.asarray(Wqkv, dtype=np.float32)
    Wout = np.asarray(Wout, dtype=np.float32)
    bout = np.asarray(bout, dtype=np.float32)
    g2 = np.asarray(g2, dtype=np.float32)

    b, c, H, W = x.shape
    xr = x.reshape(b, c, H * W)

    bf = mybir.dt.np(BF16)
    # WqkvT [c, 3H] with g1 folded per channel, [p, cb, 3H] layout
    wqkvT = np.ascontiguousarray(
        (Wqkv.T * g1[:, None]).reshape(2, 128, 3 * HID).transpose(1, 0, 2)
    ).astype(np.float32)
    woutTb = np.ascontiguousarray(
        Wout.reshape(C, HEADS, DH).transpose(2, 1, 0)).astype(np.float32)
    g2c = np.ascontiguousarray(g2.reshape(2, 128).T)
    with_bout = bool(np.any(bout))

    if with_bout not in _NC_CACHE:
        _NC_CACHE[with_bout] = build_kernel(with_bout)
    nc = _NC_CACHE[with_bout]

    in_maps = []
    for core in range(8):
        m = {
            "x": np.ascontiguousarray(xr[core * BL:(core + 1) * BL].astype(bf)),
            "wqkvT": wqkvT, "woutTb": woutTb, "g2c": g2c,
        }
        if with_bout:
            m["boutc"] = np.ascontiguousarray(bout.reshape(2, 128).T)
        in_maps.append(m)
    res = run_bass_kernel_spmd(nc, in_maps, core_ids=list(range(8)))
    out = np.concatenate(
        [np.asarray(m["out"]).astype(np.float32) for m in res.results], axis=0)
    return out.reshape(b, c, H, W)


if __name__ == "__main__":
    rng = np.random.default_rng(0)
    inputs = dict(
        x=rng.standard_normal((16, 256, 64, 64), dtype=np.float32),
        g1=np.ones(256, np.float32),
        Wqkv=(rng.standard_normal((1536, 256), dtype=np.float32) * 256 ** -0.5),
        Wout=(rng.standard_normal((256, 512), dtype=np.float32) * 512 ** -0.5),
        bout=np.zeros(256, np.float32),
        g2=np.ones(256, np.float32),
    )
    out = kernel(**inputs)
    print("out", out.shape, out.dtype, np.abs(out).max())
